# revision 1
# baseline (speedup 1.0000x reference)
"""Longformer-style windowed self-attention for TRN2, 8-core SPMD.

Sharding: 24 (batch, head) pairs -> 3 heads per core (core c gets batch c//4,
heads (c%4)*3 .. +3). Each core computes QKV projections for its head slice,
windowed attention (block 256, window +-256), and writes its [4096, 192]
output channel slice. Host gathers slices into the full [2, 4096, 768] output.

Matmuls run as float32r (full-rate fp32 path). Scores are computed
transposed ([keys, queries]) so softmax renormalization reduces over the
partition dim via a ones-column appended to V in the PV matmul; results are
PE-transposed back and divided by the row sums.
"""

import sys

for _p in ("/opt/trn_rl_repo", "/opt/pypackages"):
    if _p not in sys.path:
        sys.path.append(_p)

import numpy as np
from contextlib import ExitStack

import concourse.bass as bass
import concourse.bacc as bacc
import concourse.mybir as mybir
import concourse.tile as tile
from concourse.bass_utils import run_bass_kernel_spmd

F32 = mybir.dt.float32
R32 = mybir.dt.float32r
EXP = mybir.ActivationFunctionType.Exp

B, S, D = 2, 4096, 768
H, DH = 12, 64
W = 256                 # one-sided window / query block size
NB = S // W             # 16 query blocks
NKC = S // 128          # 32 key chunks of 128
HPC = 3                 # heads per core
N_CORES = 8


def r32(ap):
    return ap.bitcast(R32)


def _blocks_for_t(t):
    """Attention blocks whose inputs are complete after projection s-tile t."""
    if t == 0:
        return [0]
    if t == 7:
        return [13, 14, 15]
    return [2 * t - 1, 2 * t]


def build_program(has_bias, has_kmask):
    nc = bacc.Bacc("TRN2", target_bir_lowering=False, debug=False,
                   num_devices=N_CORES)
    hsT_d = nc.declare_dram_parameter("hsT", [D, S], R32, isOutput=False)
    wqk_d = nc.declare_dram_parameter("wqk", [D, 384], R32, isOutput=False)
    wv_d = nc.declare_dram_parameter("wv", [D, 256], R32, isOutput=False)
    msk_d = nc.declare_dram_parameter("masks", [128, 1024], R32, isOutput=False)
    von_d = nc.declare_dram_parameter("vones", [128, 96], R32, isOutput=False)
    idn_d = nc.declare_dram_parameter("ident", [128, 128], F32, isOutput=False)
    if has_bias:
        bqk_d = nc.declare_dram_parameter("bqk", [1, 384], R32, isOutput=False)
        bv_d = nc.declare_dram_parameter("bv", [1, 256], R32, isOutput=False)
    if has_kmask:
        kpad_d = nc.declare_dram_parameter("kpad", [128, NKC], F32, isOutput=False)
        qpad_d = nc.declare_dram_parameter("qpad", [128, NKC], F32, isOutput=False)
    out_d = nc.declare_dram_parameter("out", [S, HPC * DH], F32, isOutput=True)

    with tile.TileContext(nc) as tc, ExitStack() as ctx:
        const_p = ctx.enter_context(tc.tile_pool(name="const", bufs=1))
        hst_p = ctx.enter_context(tc.tile_pool(name="hst", bufs=3))
        qkt_p = ctx.enter_context(tc.tile_pool(name="qkt", bufs=1))
        vall_p = ctx.enter_context(tc.tile_pool(name="vall", bufs=1))
        pt_p = ctx.enter_context(tc.tile_pool(name="pt", bufs=5))
        wk_p = ctx.enter_context(tc.tile_pool(name="wk", bufs=5))
        ps_p = ctx.enter_context(tc.tile_pool(name="ps", bufs=2, space="PSUM"))
        sm_p = ctx.enter_context(tc.tile_pool(name="sm", bufs=2, space="PSUM"))

        # ---- constants / weights ----
        wqk_sb = const_p.tile([128, 6, 384], R32)
        wv_sb = const_p.tile([128, 6, 256], R32)
        nc.sync.dma_start(wqk_sb[:], wqk_d[:].rearrange("(c p) n -> p c n", p=128))
        nc.sync.dma_start(wv_sb[:], wv_d[:].rearrange("(c p) n -> p c n", p=128))
        msk_sb = const_p.tile([128, 1024], R32)
        nc.sync.dma_start(msk_sb[:], msk_d[:, :])
        idn_sb = const_p.tile([128, 128], F32)
        nc.sync.dma_start(idn_sb[:], idn_d[:, :])
        if has_bias:
            bqk_sb = const_p.tile([1, 384], R32)
            nc.sync.dma_start(bqk_sb[:], bqk_d[:, :])
            bv_sb = const_p.tile([1, 256], R32)
            nc.sync.dma_start(bv_sb[:], bv_d[:, :])
            ones_sb = const_p.tile([1, 512], R32)
            nc.vector.memset(ones_sb[:], 1.0)
        if has_kmask:
            kpad_sb = const_p.tile([128, NKC], F32)
            nc.sync.dma_start(kpad_sb[:], kpad_d[:, :])
            qpad_sb = const_p.tile([128, NKC], F32)
            nc.sync.dma_start(qpad_sb[:], qpad_d[:, :])

        # qT/kT for head pair (A,B): A on partitions 0:64, B on 64:128
        qt_ab = qkt_p.tile([128, S], R32)
        kt_ab = qkt_p.tile([128, S], R32)
        # solo head C gets base-0 tiles
        qt_c = qkt_p.tile([64, S], R32)
        kt_c = qkt_p.tile([64, S], R32)
        # v in [s, dh] layout: [128, key-chunk, (vA|1|vB|1|vC|1)]
        vall = vall_p.tile([128, NKC, 195], R32)
        ones_cols = vall[:].rearrange("p m (h x) -> p m h x", h=3)[:, :, :, 64:65]
        nc.sync.dma_start(
            ones_cols, von_d[:].rearrange("p (m h x) -> p m h x", h=3, x=1)
        )

        hst_tiles = {}

        def emit_proj_qk(t):
            s0 = 512 * t
            hst = hst_p.tile([128, 6, 512], R32)
            hst_tiles[t] = hst
            nc.sync.dma_start(
                hst[:],
                hsT_d[:].rearrange("(c p) s -> p c s", p=128)[:, :, s0 : s0 + 512],
            )
            # q/k projections: 3 pair-matmuls of M=128
            for j in range(3):
                pp = sm_p.tile([128, 512], F32, space="PSUM", tag="sm")
                for c in range(6):
                    nc.tensor.matmul(
                        pp[:],
                        (wqk_sb[:, c, 128 * j : 128 * j + 128]),
                        (hst[:, c, :]),
                        start=(c == 0),
                        stop=(c == 5 and not has_bias),
                    )
                if has_bias:
                    nc.tensor.matmul(
                        pp[:],
                        (bqk_sb[0:1, 128 * j : 128 * j + 128]),
                        (ones_sb[0:1, :]),
                        start=False,
                        stop=True,
                    )
                if j == 0:
                    nc.vector.tensor_copy(qt_ab[:, s0 : s0 + 512], pp[:])
                elif j == 1:
                    nc.vector.tensor_copy(kt_ab[:, s0 : s0 + 512], pp[:])
                else:
                    nc.vector.tensor_copy(qt_c[:, s0 : s0 + 512], pp[0:64, :])
                    scr = wk_p.tile([128, 512], R32, name="kc_scr")
                    nc.vector.tensor_copy(scr[64:128, :], pp[64:128, :])
                    nc.sync.dma_start(kt_c[:, s0 : s0 + 512], scr[64:128, :])
        def emit_proj_v(t, mms=(0, 1, 2, 3), done=True):
            s0 = 512 * t
            hst = hst_tiles.pop(t) if done else hst_tiles[t]
            # v projection: 4 s-subtiles of 128, packed two per PSUM tile
            for mm0 in mms[::2]:
                m = 4 * t + mm0
                pv = sm_p.tile([128, 512], F32, space="PSUM", tag="sm")
                for half, mm in enumerate((mm0, mm0 + 1)):
                    for c in range(6):
                        nc.tensor.matmul(
                            pv[:, 256 * half : 256 * half + 256],
                            (hst[:, c, 128 * mm : 128 * mm + 128]),
                            (wv_sb[:, c, :]),
                            start=(c == 0),
                            stop=(c == 5 and not has_bias),
                        )
                    if has_bias:
                        nc.tensor.matmul(
                            pv[:, 256 * half : 256 * half + 256],
                            (ones_sb[0:1, 0:128]),
                            (bv_sb[0:1, :]),
                            start=False,
                            stop=True,
                        )
                dst = vall[:, m : m + 2, :].rearrange(
                    "p m (h x) -> p m h x", h=3
                )[:, :, :, 0:64]
                src = pv[:].rearrange("p (m x) -> p m x", m=2)[
                    :, :, 0:192
                ].rearrange("p m (h x) -> p m h x", h=3)
                nc.vector.tensor_copy(dst, src)

        def emit_block(n):
            kept = [m for m in range(2 * n - 2, 2 * n + 4) if 0 <= m < NKC]
            j0 = kept[0] - (2 * n - 2)
            c0, c1 = 256 * j0, 256 * (j0 + len(kept))
            q0 = 256 * n

            # scores (transposed): [128 keys, 256 queries] per chunk.
            # Middle (unmasked) chunks first so exp/PV pipeline behind them.
            order = sorted(range(len(kept)), key=lambda i: abs(kept[i] - 2 * n - 0.5))

            def emit_scores(ps, kt, qt, tp):
                for i in order:
                    m = kept[i]
                    j = j0 + i
                    nc.tensor.matmul(
                        ps[:, 256 * j : 256 * j + 256],
                        (kt[:, 128 * m : 128 * m + 128]),
                        (qt[:, q0 : q0 + 256]),
                        start=True,
                        stop=True,
                        tile_position=tp,
                    )

            def emit_exp_mask(pt, ps):
                nc.scalar.activation(pt[:, c0:c1], ps[:, c0:c1], EXP)
                if n > 0:
                    nc.vector.tensor_mul(pt[:, 0:512], pt[:, 0:512], msk_sb[:, 0:512])
                if n < NB - 1:
                    nc.vector.tensor_mul(
                        pt[:, 1024:1536], pt[:, 1024:1536], msk_sb[:, 512:1024]
                    )
                if has_kmask:
                    for i, m in enumerate(kept):
                        j = j0 + i
                        nc.vector.tensor_scalar_mul(
                            pt[:, 256 * j : 256 * j + 256],
                            pt[:, 256 * j : 256 * j + 256],
                            kpad_sb[:, m : m + 1],
                        )

            ps_a = ps_p.tile([128, 1536], F32, space="PSUM", tag="ps")
            ps_b = ps_p.tile([128, 1536], F32, space="PSUM", tag="ps")
            for i in order:
                m = kept[i]
                j = j0 + i
                nc.tensor.matmul(
                    ps_a[:, 256 * j : 256 * j + 256],
                    (kt_ab[0:64, 128 * m : 128 * m + 128]),
                    (qt_ab[0:64, q0 : q0 + 256]),
                    start=True,
                    stop=True,
                    tile_position=(0, 0),
                )
                nc.tensor.matmul(
                    ps_b[:, 256 * j : 256 * j + 256],
                    (kt_ab[64:128, 128 * m : 128 * m + 128]),
                    (qt_ab[64:128, q0 : q0 + 256]),
                    start=True,
                    stop=True,
                    tile_position=(64, 0),
                )
            pt_a = pt_p.tile([128, 1536], R32, tag="pt")
            pt_b = pt_p.tile([128, 1536], R32, tag="pt")
            emit_exp_mask(pt_a, ps_a)
            emit_exp_mask(pt_b, ps_b)

            ps_c = ps_p.tile([128, 1536], F32, space="PSUM", tag="ps")
            emit_scores(ps_c, kt_c, qt_c, (0, 0))
            pt_c = pt_p.tile([128, 1536], R32, tag="pt")
            emit_exp_mask(pt_c, ps_c)

            # PV: outT_u[65, 256] per head; col 64 of lhsT is the ones column
            outp = sm_p.tile([128, 512], F32, space="PSUM", tag="sm")
            for x, pt in ((0, pt_a), (1, pt_b)):
                for oi, i in enumerate(order):
                    m = kept[i]
                    j = j0 + i
                    nc.tensor.matmul(
                        outp[0:65, 256 * x : 256 * x + 256],
                        (vall[:, m, 65 * x : 65 * x + 65]),
                        (pt[:, 256 * j : 256 * j + 256]),
                        start=(oi == 0),
                        stop=(oi == len(kept) - 1),
                    )
            ot_ab = wk_p.tile([65, 512], F32, name="ot_ab")
            nc.vector.tensor_copy(ot_ab[:], outp[0:65, :])

            outp_c = sm_p.tile([128, 512], F32, space="PSUM", tag="sm")
            for oi, i in enumerate(order):
                m = kept[i]
                j = j0 + i
                nc.tensor.matmul(
                    outp_c[0:65, 0:256],
                    (vall[:, m, 130:195]),
                    (pt_c[:, 256 * j : 256 * j + 256]),
                    start=(oi == 0),
                    stop=(oi == len(kept) - 1),
                )

            # transpose [65, 128] -> [128, 65]; col 64 holds the softmax denom
            trp = sm_p.tile([128, 512], F32, space="PSUM", tag="sm")
            for x in range(2):
                for g in range(2):
                    i4 = 2 * x + g
                    nc.tensor.transpose(
                        trp[:, 65 * i4 : 65 * i4 + 65],
                        ot_ab[0:65, 256 * x + 128 * g : 256 * x + 128 * g + 128],
                        idn_sb[0:65, 0:65],
                    )
            ot_c = wk_p.tile([65, 256], F32, name="ot_c")
            nc.vector.tensor_copy(ot_c[:], outp_c[0:65, 0:256])

            dcol = trp[:, 0:260].rearrange("p (i x) -> p i x", x=65)[:, :, 64:65]
            rec = wk_p.tile([128, 4], F32, name="rec")
            nc.vector.reciprocal(rec[:].rearrange("p (i x) -> p i x", x=1), dcol)
            for g in range(2):
                osb = wk_p.tile([128, 128], F32, name="osb")
                for x in range(2):
                    i4 = 2 * x + g
                    nc.vector.tensor_scalar_mul(
                        osb[:, 64 * x : 64 * x + 64],
                        trp[:, 65 * i4 : 65 * i4 + 64],
                        rec[:, i4 : i4 + 1],
                    )
                if has_kmask:
                    nc.vector.tensor_scalar_mul(
                        osb[:], osb[:], qpad_sb[:, 2 * n + g : 2 * n + g + 1]
                    )
                nc.sync.dma_start(
                    out_d[q0 + 128 * g : q0 + 128 * g + 128, 0:128], osb[:]
                )

            trp_c = sm_p.tile([128, 512], F32, space="PSUM", tag="sm")
            for g in range(2):
                nc.tensor.transpose(
                    trp_c[:, 65 * g : 65 * g + 65],
                    ot_c[0:65, 128 * g : 128 * g + 128],
                    idn_sb[0:65, 0:65],
                )
            dcol_c = trp_c[:, 0:130].rearrange("p (i x) -> p i x", x=65)[:, :, 64:65]
            rec_c = wk_p.tile([128, 2], F32, name="rec_c")
            nc.vector.reciprocal(rec_c[:].rearrange("p (i x) -> p i x", x=1), dcol_c)
            for g in range(2):
                osb_c = wk_p.tile([128, 64], F32, name="osb_c")
                nc.vector.tensor_scalar_mul(
                    osb_c[:], trp_c[:, 65 * g : 65 * g + 64], rec_c[:, g : g + 1]
                )
                if has_kmask:
                    nc.vector.tensor_scalar_mul(
                        osb_c[:], osb_c[:], qpad_sb[:, 2 * n + g : 2 * n + g + 1]
                    )
                nc.sync.dma_start(
                    out_d[q0 + 128 * g : q0 + 128 * g + 128, 128:192], osb_c[:]
                )

        # Interleave: a projection s-tile between attention blocks keeps PE
        # busy while the previous block's exp/mask/epilogue chains drain.
        def emit_proj(t):
            emit_proj_qk(t)
            emit_proj_v(t)

        emit_proj(0)
        emit_proj(1)
        emit_block(0)
        nb_next = 1
        for t in range(2, 8):
            emit_proj(t)
            emit_block(nb_next)
            emit_block(nb_next + 1)
            nb_next += 2
        for n in range(nb_next, NB):
            emit_block(n)

    nc.compile()
    return nc


_prog_cache = {}


def _get_program(has_bias, has_kmask):
    key = (has_bias, has_kmask)
    if key not in _prog_cache:
        _prog_cache[key] = build_program(has_bias, has_kmask)
    return _prog_cache[key]


def _band_masks():
    """Multiplicative band masks for window chunks 0,1,4,5: [128, 4*256]."""
    r = np.arange(128)[:, None]
    q = np.arange(256)[None, :]
    m0 = (q <= r).astype(np.float32)
    m1 = (q <= r + 128).astype(np.float32)
    m4 = (r <= q).astype(np.float32)
    m5 = (r + 128 <= q).astype(np.float32)
    return np.concatenate([m0, m1, m4, m5], axis=1)


def kernel(hidden_states, attention_mask, Wq, bq, Wk, bk, Wv, bv, _res=[None]):
    hidden_states = np.asarray(hidden_states, np.float32)
    attention_mask = np.asarray(attention_mask, np.float32)
    Wq, Wk, Wv = (np.asarray(w, np.float32) for w in (Wq, Wk, Wv))
    bq, bk, bv = (np.asarray(b_, np.float32) for b_ in (bq, bk, bv))

    scale = 1.0 / np.sqrt(DH)
    has_bias = bool(np.any(bq) or np.any(bk) or np.any(bv))
    has_kmask = bool(np.any(attention_mask < 0))

    hsT = [np.ascontiguousarray(hidden_states[b].T) for b in range(B)]
    masks = _band_masks()
    ident = np.eye(128, dtype=np.float32)
    masked = attention_mask < 0  # [B, S]

    in_maps = []
    for core in range(N_CORES):
        b, h0 = core // 4, (core % 4) * HPC
        sl = slice(h0 * DH, (h0 + HPC) * DH)
        wq = Wq[:, sl] * scale
        wk = Wk[:, sl]
        wqk = np.concatenate(
            [wq[:, 0:128], wk[:, 0:128], wq[:, 128:192], wk[:, 128:192]], axis=1
        )
        wv = np.zeros((D, 256), np.float32)
        wv[:, 0:192] = Wv[:, sl]
        m = {
            "hsT": hsT[b],
            "wqk": np.ascontiguousarray(wqk),
            "wv": wv,
            "masks": masks,
            "vones": np.ones((128, 96), np.float32),
            "ident": ident,
        }
        if has_bias:
            bq_s = bq[sl] * scale
            bk_s = bk[sl]
            m["bqk"] = np.concatenate(
                [bq_s[0:128], bk_s[0:128], bq_s[128:192], bk_s[128:192]]
            ).reshape(1, 384).astype(np.float32)
            bvp = np.zeros((1, 256), np.float32)
            bvp[0, 0:192] = bv[sl]
            m["bv"] = bvp
        if has_kmask:
            keep = (~masked[b]).astype(np.float32).reshape(NKC, 128).T
            m["kpad"] = np.ascontiguousarray(keep)
            m["qpad"] = np.ascontiguousarray(keep)
        in_maps.append(m)

    nc = _get_program(has_bias, has_kmask)
    res = run_bass_kernel_spmd(nc, in_maps, list(range(N_CORES)))
    _res[0] = res

    out = np.empty((B, S, D), np.float32)
    for core in range(N_CORES):
        b, h0 = core // 4, (core % 4) * HPC
        out[b, :, h0 * DH : (h0 + HPC) * DH] = res.results[core]["out"]
    return out



# revision 8
# speedup vs baseline: 1.1701x; 1.1701x over previous
"""Longformer-style windowed self-attention for TRN2, 8-core SPMD.

Sharding: 24 (batch, head) pairs -> 3 heads per core (core c gets batch c//4,
heads (c%4)*3 .. +3). Each core computes QKV projections for its head slice,
windowed attention (block 256, window +-256), and writes its [4096, 192]
output channel slice. Host gathers slices into the full [2, 4096, 768] output.

All matmul inputs are bf16 (psum accumulation fp32). Scores are computed
transposed ([keys, queries]); the softmax window is trimmed: the two outer
key chunks of each 6-chunk window only touch the 128-query half they can
reach, so each head-block does 1280 score columns instead of 1536. Per-head
psum score layout packs chunks as [c1|c0|c5][c4|c2][c3] so the four masked
chunks form one contiguous 768-column region (one multiply on GpSimd) and
exp covers one contiguous 1280-column region (one Activation op).
Renormalization reduces over the partition dim via a ones-column appended to
V; results are PE-transposed back and scaled by the reciprocal row sums.
"""

import sys

for _p in ("/opt/trn_rl_repo", "/opt/pypackages"):
    if _p not in sys.path:
        sys.path.append(_p)

import numpy as np
import ml_dtypes
from contextlib import ExitStack

import concourse.bass as bass
import concourse.bacc as bacc
import concourse.mybir as mybir
import concourse.tile as tile
from concourse.bass_utils import run_bass_kernel_spmd

F32 = mybir.dt.float32
BF16 = mybir.dt.bfloat16
EXP = mybir.ActivationFunctionType.Exp
BF = ml_dtypes.bfloat16

B, S, D = 2, 4096, 768
H, DH = 12, 64
W = 256                 # one-sided window / query block size
NB = S // W             # 16 query blocks
NKC = S // 128          # 32 key chunks of 128
HPC = 3                 # heads per core
N_CORES = 8


def _ab_layout(n):
    """Per-block score-psum layout: list of (ci, m, col, width, qoff) plus
    exp ranges and mask ops. ci = chunk position in the 6-chunk window
    (m = 2n-2+ci), col = psum column, width = query count, qoff = query
    offset within the 256-query block (half-blocks only).

    Full blocks: [c1 | c0 | c5] [c4 | c2] [c3] -> masked chunks contiguous
    at [0:768], exp range [0:1280], no psum bank crossing.
    """
    if 0 < n < NB - 1:
        chunks = [
            (1, 0, 256, 0),
            (0, 256, 128, 0),
            (5, 384, 128, 128),
            (4, 512, 256, 0),
            (2, 768, 256, 0),
            (3, 1024, 256, 0),
        ]
        exp_ranges = [(0, 1280)]
        # mskAB = [m1 | m0 | m5 | m4] matches [c1 | c0 | c5 | c4] directly
        mask_ops = [(0, 768, 0)]
    elif n == 0:
        chunks = [
            (4, 0, 256, 0),
            (5, 256, 128, 128),
            (2, 512, 256, 0),
            (3, 768, 256, 0),
        ]
        exp_ranges = [(0, 384), (512, 512)]
        mask_ops = [(0, 256, 512), (256, 128, 384)]  # c4 <- m4, c5 <- m5
    else:  # n == NB - 1
        chunks = [
            (1, 0, 256, 0),
            (0, 256, 128, 0),
            (2, 512, 256, 0),
            (3, 768, 256, 0),
        ]
        exp_ranges = [(0, 384), (512, 512)]
        mask_ops = [(0, 384, 0)]  # [c1 | c0] <- [m1 | m0]
    chunks = [(ci, 2 * n - 2 + ci, col, w_, qo) for ci, col, w_, qo in chunks]
    return chunks, exp_ranges, mask_ops


def build_program(has_bias, has_kmask):
    nc = bacc.Bacc("TRN2", target_bir_lowering=False, debug=False,
                   num_devices=N_CORES)
    hsT_d = nc.declare_dram_parameter("hsT", [D, S], BF16, isOutput=False)
    wqk_d = nc.declare_dram_parameter("wqk", [D, 384], BF16, isOutput=False)
    wv_d = nc.declare_dram_parameter("wv", [D, 192], BF16, isOutput=False)
    msk_d = nc.declare_dram_parameter("masks", [128, 768], BF16, isOutput=False)
    von_d = nc.declare_dram_parameter("vones", [128, 96], BF16, isOutput=False)
    idn_d = nc.declare_dram_parameter("ident", [128, 128], BF16, isOutput=False)
    if has_bias:
        bqk_d = nc.declare_dram_parameter("bqk", [1, 384], BF16, isOutput=False)
        bv_d = nc.declare_dram_parameter("bv", [1, 192], BF16, isOutput=False)
    if has_kmask:
        kpad_d = nc.declare_dram_parameter("kpad", [128, NKC], F32, isOutput=False)
        qpad_d = nc.declare_dram_parameter("qpad", [128, NKC], F32, isOutput=False)
    out_d = nc.declare_dram_parameter("out", [S, HPC * DH], F32, isOutput=True)

    with tile.TileContext(nc) as tc, ExitStack() as ctx:
        const_p = ctx.enter_context(tc.tile_pool(name="const", bufs=1))
        hst_p = ctx.enter_context(tc.tile_pool(name="hst", bufs=3))
        qkt_p = ctx.enter_context(tc.tile_pool(name="qkt", bufs=1))
        vall_p = ctx.enter_context(tc.tile_pool(name="vall", bufs=1))
        pt_p = ctx.enter_context(tc.tile_pool(name="pt", bufs=6))
        wk_p = ctx.enter_context(tc.tile_pool(name="wk", bufs=6))
        ps_p = ctx.enter_context(tc.tile_pool(name="ps", bufs=2, space="PSUM"))
        sm_p = ctx.enter_context(tc.tile_pool(name="sm", bufs=2, space="PSUM"))

        # ---- constants / weights ----
        wqk_sb = const_p.tile([128, 6, 384], BF16)
        wv_sb = const_p.tile([128, 6, 192], BF16)
        nc.sync.dma_start(wqk_sb[:], wqk_d[:].rearrange("(c p) n -> p c n", p=128))
        nc.sync.dma_start(wv_sb[:], wv_d[:].rearrange("(c p) n -> p c n", p=128))
        msk_sb = const_p.tile([128, 768], BF16)
        nc.sync.dma_start(msk_sb[:], msk_d[:, :])
        idn_sb = const_p.tile([128, 128], BF16)
        nc.sync.dma_start(idn_sb[:], idn_d[:, :])
        if has_bias:
            bqk_sb = const_p.tile([1, 384], BF16)
            nc.sync.dma_start(bqk_sb[:], bqk_d[:, :])
            bv_sb = const_p.tile([1, 192], BF16)
            nc.sync.dma_start(bv_sb[:], bv_d[:, :])
            ones_sb = const_p.tile([1, 512], BF16)
            nc.vector.memset(ones_sb[:], 1.0)
        if has_kmask:
            kpad_sb = const_p.tile([128, NKC], F32)
            nc.sync.dma_start(kpad_sb[:], kpad_d[:, :])
            qpad_sb = const_p.tile([128, NKC], F32)
            nc.sync.dma_start(qpad_sb[:], qpad_d[:, :])

        # qT/kT for head pair (A,B): A on partitions 0:64, B on 64:128.
        # Head C: qkt_c holds qC on 0:64 / kC on 64:128; qkt_c2[0:64] is a
        # DMA-replicated copy of kC so both score operands sit on 0:64.
        qt_ab = qkt_p.tile([128, S], BF16)
        kt_ab = qkt_p.tile([128, S], BF16)
        qkt_c = qkt_p.tile([128, S], BF16)
        qkt_c2 = qkt_p.tile([64, S], BF16)
        # v in [s, dh] layout: [128, key-chunk, (vA|1|vB|1|vC|1)]
        vall = vall_p.tile([128, NKC, 195], BF16)
        ones_cols = vall[:].rearrange("p m (h x) -> p m h x", h=3)[:, :, :, 64:65]
        nc.sync.dma_start(
            ones_cols, von_d[:].rearrange("p (m h x) -> p m h x", h=3, x=1)
        )

        hst_tiles = {}

        def emit_proj(t):
            s0 = 512 * t
            hst = hst_p.tile([128, 6, 512], BF16)
            hst_tiles[t] = hst
            nc.sync.dma_start(
                hst[:],
                hsT_d[:].rearrange("(c p) s -> p c s", p=128)[:, :, s0 : s0 + 512],
            )
            # q/k projections: 3 pair-matmuls of M=128 -> [qA|qB], [kA|kB],
            # [qC|kC]
            for j in range(3):
                pp = sm_p.tile([128, 512], F32, space="PSUM", tag="sm")
                for c in range(6):
                    nc.tensor.matmul(
                        pp[:],
                        wqk_sb[:, c, 128 * j : 128 * j + 128],
                        hst[:, c, :],
                        start=(c == 0),
                        stop=(c == 5 and not has_bias),
                    )
                if has_bias:
                    nc.tensor.matmul(
                        pp[:],
                        bqk_sb[0:1, 128 * j : 128 * j + 128],
                        ones_sb[0:1, :],
                        start=False,
                        stop=True,
                    )
                dst = (qt_ab, kt_ab, qkt_c)[j]
                nc.vector.tensor_copy(dst[:, s0 : s0 + 512], pp[:])
            nc.sync.dma_start(qkt_c2[:, s0 : s0 + 512], qkt_c[64:128, s0 : s0 + 512])
            # v projection: 4 s-subtiles of 128, packed two per PSUM tile
            for mm0 in (0, 2):
                m = 4 * t + mm0
                pv = sm_p.tile([128, 512], F32, space="PSUM", tag="sm")
                for half, mm in enumerate((mm0, mm0 + 1)):
                    for c in range(6):
                        nc.tensor.matmul(
                            pv[:, 256 * half : 256 * half + 192],
                            hst[:, c, 128 * mm : 128 * mm + 128],
                            wv_sb[:, c, :],
                            start=(c == 0),
                            stop=(c == 5 and not has_bias),
                        )
                    if has_bias:
                        nc.tensor.matmul(
                            pv[:, 256 * half : 256 * half + 192],
                            ones_sb[0:1, 0:128],
                            bv_sb[0:1, :],
                            start=False,
                            stop=True,
                        )
                dst = vall[:, m : m + 2, :].rearrange(
                    "p m (h x) -> p m h x", h=3
                )[:, :, :, 0:64]
                src = pv[:].rearrange("p (m x) -> p m x", m=2)[
                    :, :, 0:192
                ].rearrange("p m (h x) -> p m h x", h=3)
                nc.vector.tensor_copy(dst, src)

        # state passed from emit_scores(n) to emit_pv(n)
        blk = {}

        def emit_scores(n):
            q0 = 256 * n
            chunks, exp_ranges, mask_ops = _ab_layout(n)
            pts = []
            for h in range(3):
                if h == 0:
                    kt, qt, p0 = kt_ab, qt_ab, 0
                elif h == 1:
                    kt, qt, p0 = kt_ab, qt_ab, 64
                else:
                    kt, qt, p0 = qkt_c2, qkt_c, 0
                ps = ps_p.tile([128, 1536], F32, space="PSUM", tag="ps")
                for ci, m, col, w_, qo in chunks:
                    nc.tensor.matmul(
                        ps[:, col : col + w_],
                        kt[p0 : p0 + 64, 128 * m : 128 * m + 128],
                        qt[p0 : p0 + 64, q0 + qo : q0 + qo + w_],
                        start=True,
                        stop=True,
                        tile_position=(p0, 0),
                    )
                pt = pt_p.tile([128, 1536], BF16, tag="pt")
                for a, ln in exp_ranges:
                    nc.scalar.activation(pt[:, a : a + ln], ps[:, a : a + ln], EXP)
                for a, ln, moff in mask_ops:
                    nc.vector.tensor_mul(
                        pt[:, a : a + ln], pt[:, a : a + ln],
                        msk_sb[:, moff : moff + ln],
                    )
                if has_kmask:
                    for ci, m, col, w_, qo in chunks:
                        nc.vector.tensor_scalar_mul(
                            pt[:, col : col + w_],
                            pt[:, col : col + w_],
                            kpad_sb[:, m : m + 1],
                        )
                pts.append(pt)
            blk[n] = (pts, chunks)

        def emit_pv(n):
            q0 = 256 * n
            pts, chunks = blk.pop(n)
            # order: first full-width chunk starts the psum group, last one
            # stops it; half-width chunks accumulate in between.
            full = [c for c in chunks if c[3] == 256]
            halves = [c for c in chunks if c[3] != 256]
            order = [full[0]] + halves + full[1:]
            last_full = len(order) - 1

            pvAB = sm_p.tile([128, 512], F32, space="PSUM", tag="sm")
            pvC = sm_p.tile([128, 512], F32, space="PSUM", tag="sm")
            for h in range(3):
                for oi, (ci, m, col, w_, qo) in enumerate(order):
                    if h < 2:
                        od = pvAB[0:65, 256 * h + qo : 256 * h + qo + w_]
                    else:
                        od = pvC[0:65, qo : qo + w_]
                    nc.tensor.matmul(
                        od,
                        vall[:, m, 65 * h : 65 * h + 65],
                        pts[h][:, col : col + w_],
                        start=(oi == 0),
                        stop=(oi == last_full),
                        skip_group_check=True,
                    )

            ot_ab = wk_p.tile([65, 512], BF16, name="ot_ab")
            nc.vector.tensor_copy(ot_ab[:], pvAB[0:65, :])
            ot_c = wk_p.tile([65, 256], BF16, name="ot_c")
            nc.vector.tensor_copy(ot_c[:], pvC[0:65, 0:256])

            # transpose [65, 128] -> [128, 65] per (head, query-half);
            # col 64 of each 65-group holds the softmax denominator.
            trp = ps_p.tile([128, 512], BF16, space="PSUM", tag="ps")
            for g in range(2):
                for h in range(3):
                    src = ot_ab[0:65, 256 * h + 128 * g : 256 * h + 128 * g + 128] \
                        if h < 2 else ot_c[0:65, 128 * g : 128 * g + 128]
                    nc.tensor.transpose(
                        trp[:, 256 * g + 66 * h : 256 * g + 66 * h + 65],
                        src,
                        idn_sb[0:65, 0:65],
                    )
            rec = wk_p.tile([128, 8], F32, name="rec")
            for g in range(2):
                dcol = trp[:, 256 * g : 256 * g + 198].rearrange(
                    "p (i x) -> p i x", x=66
                )[:, :, 64:65]
                nc.vector.reciprocal(
                    rec[:, 4 * g : 4 * g + 3].rearrange("p (i x) -> p i x", x=1),
                    dcol,
                )
            for g in range(2):
                osb = wk_p.tile([128, 192], F32, name="osb")
                for h in range(3):
                    nc.vector.tensor_scalar_mul(
                        osb[:, 64 * h : 64 * h + 64],
                        trp[:, 256 * g + 66 * h : 256 * g + 66 * h + 64],
                        rec[:, 4 * g + h : 4 * g + h + 1],
                    )
                if has_kmask:
                    nc.vector.tensor_scalar_mul(
                        osb[:], osb[:], qpad_sb[:, 2 * n + g : 2 * n + g + 1]
                    )
                nc.sync.dma_start(
                    out_d[q0 + 128 * g : q0 + 128 * g + 128, 0:192], osb[:]
                )

        # Software pipeline: scores(n) on PE while exp/mask(n-1) drain on
        # Act/Pool, then PV+epilogue(n-1); projection s-tiles interleave.
        emit_proj(0)
        emit_proj(1)
        for i in range(NB + 1):
            if i >= 2 and i % 2 == 0 and i // 2 + 1 <= 7:
                emit_proj(i // 2 + 1)
            if i < NB:
                emit_scores(i)
            if i >= 1:
                emit_pv(i - 1)

    nc.compile()
    return nc


_prog_cache = {}


def _get_program(has_bias, has_kmask):
    key = (has_bias, has_kmask)
    if key not in _prog_cache:
        _prog_cache[key] = build_program(has_bias, has_kmask)
    return _prog_cache[key]


def _band_masks():
    """[m1 | m0 | m5 | m4] multiplicative band masks, [128, 768].

    In [key-row r, query-col j] space: m1 keeps j <= r+128 (256 wide),
    m0 keeps j <= r (128), m5 keeps j >= r (128), m4 keeps j >= r (256).
    """
    r = np.arange(128)[:, None]
    q256 = np.arange(256)[None, :]
    q128 = np.arange(128)[None, :]
    m1 = (q256 <= r + 128).astype(np.float32)
    m0 = (q128 <= r).astype(np.float32)
    m5 = (q128 >= r).astype(np.float32)
    m4 = (q256 >= r).astype(np.float32)
    return np.concatenate([m1, m0, m5, m4], axis=1)


def kernel(hidden_states, attention_mask, Wq, bq, Wk, bk, Wv, bv, _res=[None]):
    hidden_states = np.asarray(hidden_states, np.float32)
    attention_mask = np.asarray(attention_mask, np.float32)
    Wq, Wk, Wv = (np.asarray(w, np.float32) for w in (Wq, Wk, Wv))
    bq, bk, bv = (np.asarray(b_, np.float32) for b_ in (bq, bk, bv))

    scale = 1.0 / np.sqrt(DH)
    has_bias = bool(np.any(bq) or np.any(bk) or np.any(bv))
    has_kmask = bool(np.any(attention_mask < 0))

    hsT = [np.ascontiguousarray(hidden_states[b].T).astype(BF) for b in range(B)]
    masks = _band_masks().astype(BF)
    ident = np.eye(128, dtype=np.float32).astype(BF)
    vones = np.ones((128, 96), BF)
    masked = attention_mask < 0  # [B, S]

    in_maps = []
    for core in range(N_CORES):
        b, h0 = core // 4, (core % 4) * HPC
        sl = slice(h0 * DH, (h0 + HPC) * DH)
        wq = Wq[:, sl] * scale
        wk = Wk[:, sl]
        wqk = np.concatenate(
            [wq[:, 0:128], wk[:, 0:128], wq[:, 128:192], wk[:, 128:192]], axis=1
        )
        m = {
            "hsT": hsT[b],
            "wqk": np.ascontiguousarray(wqk).astype(BF),
            "wv": np.ascontiguousarray(Wv[:, sl]).astype(BF),
            "masks": masks,
            "vones": vones,
            "ident": ident,
        }
        if has_bias:
            bq_s = bq[sl] * scale
            bk_s = bk[sl]
            m["bqk"] = np.concatenate(
                [bq_s[0:128], bk_s[0:128], bq_s[128:192], bk_s[128:192]]
            ).reshape(1, 384).astype(BF)
            m["bv"] = bv[sl].reshape(1, 192).astype(BF)
        if has_kmask:
            keep = (~masked[b]).astype(np.float32).reshape(NKC, 128).T
            m["kpad"] = np.ascontiguousarray(keep)
            m["qpad"] = np.ascontiguousarray(keep)
        in_maps.append(m)

    nc = _get_program(has_bias, has_kmask)
    res = run_bass_kernel_spmd(nc, in_maps, list(range(N_CORES)))
    _res[0] = res

    out = np.empty((B, S, D), np.float32)
    for core in range(N_CORES):
        b, h0 = core // 4, (core % 4) * HPC
        out[b, :, h0 * DH : (h0 + HPC) * DH] = res.results[core]["out"]
    return out


# revision 14
# speedup vs baseline: 1.2272x; 1.0488x over previous
"""Longformer-style windowed self-attention for TRN2, 8-core SPMD.

Sharding: 24 (batch, head) pairs -> 3 heads per core (core c gets batch c//4,
heads (c%4)*3 .. +3). Each core computes QKV projections for its head slice,
windowed attention (block 256, window +-256), and writes its [4096, 192]
output channel slice. Host gathers slices into the full [2, 4096, 768] output.

All matmul inputs are bf16 (psum accumulation fp32). Scores are computed
transposed ([keys, queries]); the softmax window is trimmed: the two outer
key chunks of each 6-chunk window only touch the 128-query half they can
reach, so each head-block does 1280 score columns instead of 1536. Per-head
psum score layout packs chunks as [c1|c0|c5][c4|c2][c3] so the four masked
chunks form one contiguous 768-column region (one multiply on GpSimd) and
exp covers one contiguous 1280-column region (one Activation op).
Renormalization reduces over the partition dim via a ones-column appended to
V; results are PE-transposed back and scaled by the reciprocal row sums.
"""

import sys

for _p in ("/opt/trn_rl_repo", "/opt/pypackages"):
    if _p not in sys.path:
        sys.path.append(_p)

import numpy as np
import ml_dtypes
from contextlib import ExitStack

import concourse.bass as bass
import concourse.bacc as bacc
import concourse.mybir as mybir
import concourse.tile as tile
from concourse.bass_utils import run_bass_kernel_spmd

F32 = mybir.dt.float32
BF16 = mybir.dt.bfloat16
EXP = mybir.ActivationFunctionType.Exp
BF = ml_dtypes.bfloat16

B, S, D = 2, 4096, 768
H, DH = 12, 64
W = 256                 # one-sided window / query block size
NB = S // W             # 16 query blocks
NKC = S // 128          # 32 key chunks of 128
HPC = 3                 # heads per core
N_CORES = 8


NSB = S // 512          # 8 query superblocks of 512

# psum column of chunk i within its piece (piece 0: i<=3, piece 1: i>=4)
_SB_COL = {2: 0, 0: 384, 3: 512, 1: 1024, 4: 0, 5: 512, 7: 896, 6: 1024}


def _sb_chunks(s):
    """Superblock s covers queries [512s, 512s+512); its key window is the
    8 chunks m = 4s-2 .. 4s+5 (chunk position i = m - 4s + 2). Chunk i is
    valid for superblock-relative queries [max(0, 128(i-4)), min(512,
    128(i+1))) — extents 128/256/384/512/512/384/256/128. Left chunks
    (i<=3) are diagonal-masked on the last 128 columns of their extent
    (keep j <= r), right chunks (i>=4) on the first 128 (keep j >= r).

    Returns [(i, m, piece, col, width, qlo)].
    """
    out = []
    for i in range(8):
        m = 4 * s - 2 + i
        if not (0 <= m < NKC):
            continue
        qlo = max(0, 128 * (i - 4))
        qhi = min(512, 128 * (i + 1))
        out.append((i, m, i // 4, _SB_COL[i], qhi - qlo, qlo))
    return out


def _merge_ranges(ivals):
    """Merge sorted [start, end) col intervals into contiguous runs."""
    ivals = sorted(ivals)
    out = [list(ivals[0])]
    for a, b_ in ivals[1:]:
        if a == out[-1][1]:
            out[-1][1] = b_
        else:
            out.append([a, b_])
    return [(a, b_ - a) for a, b_ in out]


def build_program(has_bias, has_kmask):
    nc = bacc.Bacc("TRN2", target_bir_lowering=False, debug=False,
                   num_devices=N_CORES)
    hsT_d = nc.declare_dram_parameter("hsT", [D, S], BF16, isOutput=False)
    wqk_d = nc.declare_dram_parameter("wqk", [D, 384], BF16, isOutput=False)
    wv_d = nc.declare_dram_parameter("wv", [D, 192], BF16, isOutput=False)
    msk_d = nc.declare_dram_parameter("masks", [128, 512], BF16, isOutput=False)
    von_d = nc.declare_dram_parameter("vones", [128, 96], BF16, isOutput=False)
    idn_d = nc.declare_dram_parameter("ident", [128, 128], BF16, isOutput=False)
    if has_bias:
        bqk_d = nc.declare_dram_parameter("bqk", [1, 384], BF16, isOutput=False)
        bv_d = nc.declare_dram_parameter("bv", [1, 192], BF16, isOutput=False)
    if has_kmask:
        kpad_d = nc.declare_dram_parameter("kpad", [128, NKC], F32, isOutput=False)
        qpad_d = nc.declare_dram_parameter("qpad", [128, NKC], F32, isOutput=False)
    out_d = nc.declare_dram_parameter("out", [S, HPC * DH], F32, isOutput=True)

    with tile.TileContext(nc) as tc, ExitStack() as ctx:
        const_p = ctx.enter_context(tc.tile_pool(name="const", bufs=1))
        hst_p = ctx.enter_context(tc.tile_pool(name="hst", bufs=3))
        qkt_p = ctx.enter_context(tc.tile_pool(name="qkt", bufs=1))
        vall_p = ctx.enter_context(tc.tile_pool(name="vall", bufs=1))
        pt_p = ctx.enter_context(tc.tile_pool(name="pt", bufs=6))
        wk_p = ctx.enter_context(tc.tile_pool(name="wk", bufs=6))
        ps_p = ctx.enter_context(tc.tile_pool(name="ps", bufs=2, space="PSUM"))
        sm_p = ctx.enter_context(tc.tile_pool(name="sm", bufs=2, space="PSUM"))

        # ---- constants / weights ----
        wqk_sb = const_p.tile([128, 6, 384], BF16)
        wv_sb = const_p.tile([128, 6, 192], BF16)
        nc.sync.dma_start(wqk_sb[:], wqk_d[:].rearrange("(c p) n -> p c n", p=128))
        nc.sync.dma_start(wv_sb[:], wv_d[:].rearrange("(c p) n -> p c n", p=128))
        msk_sb = const_p.tile([128, 512], BF16)
        nc.sync.dma_start(msk_sb[:], msk_d[:, :])
        idn_sb = const_p.tile([128, 128], BF16)
        nc.sync.dma_start(idn_sb[:], idn_d[:, :])
        if has_bias:
            bqk_sb = const_p.tile([1, 384], BF16)
            nc.sync.dma_start(bqk_sb[:], bqk_d[:, :])
            bv_sb = const_p.tile([1, 192], BF16)
            nc.sync.dma_start(bv_sb[:], bv_d[:, :])
            ones_sb = const_p.tile([1, 512], BF16)
            nc.vector.memset(ones_sb[:], 1.0)
        if has_kmask:
            kpad_sb = const_p.tile([128, NKC], F32)
            nc.sync.dma_start(kpad_sb[:], kpad_d[:, :])
            qpad_sb = const_p.tile([128, NKC], F32)
            nc.sync.dma_start(qpad_sb[:], qpad_d[:, :])

        # qT/kT for head pair (A,B): A on partitions 0:64, B on 64:128.
        # Head C: qkt_c holds qC on 0:64 / kC on 64:128; qkt_c2[0:64] is a
        # DMA-replicated copy of kC so both score operands sit on 0:64.
        qt_ab = qkt_p.tile([128, S], BF16)
        kt_ab = qkt_p.tile([128, S], BF16)
        qkt_c = qkt_p.tile([128, S], BF16)
        qkt_c2 = qkt_p.tile([64, S], BF16)
        # v in [s, dh] layout: [128, key-chunk, (vA|1|vB|1|vC|1)]
        vall = vall_p.tile([128, NKC, 195], BF16)
        ones_cols = vall[:].rearrange("p m (h x) -> p m h x", h=3)[:, :, :, 64:65]
        nc.sync.dma_start(
            ones_cols, von_d[:].rearrange("p (m h x) -> p m h x", h=3, x=1)
        )

        hst_tiles = {}

        def emit_proj(t):
            s0 = 512 * t
            hst = hst_p.tile([128, 6, 512], BF16)
            hst_tiles[t] = hst
            nc.sync.dma_start(
                hst[:],
                hsT_d[:].rearrange("(c p) s -> p c s", p=128)[:, :, s0 : s0 + 512],
            )
            # q/k projections: 3 pair-matmuls of M=128 -> [qA|qB], [kA|kB],
            # [qC|kC]
            for j in range(3):
                pp = sm_p.tile([128, 512], F32, space="PSUM", tag="sm")
                for c in range(6):
                    nc.tensor.matmul(
                        pp[:],
                        wqk_sb[:, c, 128 * j : 128 * j + 128],
                        hst[:, c, :],
                        start=(c == 0),
                        stop=(c == 5 and not has_bias),
                    )
                if has_bias:
                    nc.tensor.matmul(
                        pp[:],
                        bqk_sb[0:1, 128 * j : 128 * j + 128],
                        ones_sb[0:1, :],
                        start=False,
                        stop=True,
                    )
                dst = (qt_ab, kt_ab, qkt_c)[j]
                nc.vector.tensor_copy(dst[:, s0 : s0 + 512], pp[:])
            nc.sync.dma_start(qkt_c2[:, s0 : s0 + 512], qkt_c[64:128, s0 : s0 + 512])
            # v projection: 4 s-subtiles of 128, packed two per PSUM tile
            for mm0 in (0, 2):
                m = 4 * t + mm0
                pv = sm_p.tile([128, 512], F32, space="PSUM", tag="sm")
                for half, mm in enumerate((mm0, mm0 + 1)):
                    for c in range(6):
                        nc.tensor.matmul(
                            pv[:, 256 * half : 256 * half + 192],
                            hst[:, c, 128 * mm : 128 * mm + 128],
                            wv_sb[:, c, :],
                            start=(c == 0),
                            stop=(c == 5 and not has_bias),
                        )
                    if has_bias:
                        nc.tensor.matmul(
                            pv[:, 256 * half : 256 * half + 192],
                            ones_sb[0:1, 0:128],
                            bv_sb[0:1, :],
                            start=False,
                            stop=True,
                        )
                dst = vall[:, m : m + 2, :].rearrange(
                    "p m (h x) -> p m h x", h=3
                )[:, :, :, 0:64]
                src = pv[:].rearrange("p (m x) -> p m x", m=2)[
                    :, :, 0:192
                ].rearrange("p m (h x) -> p m h x", h=3)
                nc.vector.tensor_copy(dst, src)

        def emit_mask(pt, in_off, nreg, stride, msk_off):
            """pt[:, in_off + k*stride : +128] *= msk[:, msk_off + k*128]
            for k in range(nreg), as one strided TensorTensor."""
            if nreg == 1:
                in_ap = pt[:, in_off : in_off + 128]
                mk_ap = msk_sb[:, msk_off : msk_off + 128]
            else:
                ln = stride * (nreg - 1) + 128
                in_ap = pt[:, in_off : in_off + ln].rearrange(
                    "p (a x) -> p a x", x=128
                )[:, :: stride // 128, :]
                mk_ap = msk_sb[:, msk_off : msk_off + 128 * nreg].rearrange(
                    "p (a x) -> p a x", x=128
                )
            nc.gpsimd.tensor_mul(in_ap, in_ap, mk_ap)

        # state passed from emit_scores(s) to emit_pv(s)
        blk = {}

        def emit_scores(s):
            q0 = 512 * s
            chunks = _sb_chunks(s)
            pts = []
            for h in range(3):
                if h == 0:
                    kt, qt, p0 = kt_ab, qt_ab, 0
                elif h == 1:
                    kt, qt, p0 = kt_ab, qt_ab, 64
                else:
                    kt, qt, p0 = qkt_c2, qkt_c, 0
                hpt = []
                for piece in range(2):
                    pc = [c for c in chunks if c[2] == piece]
                    ps = ps_p.tile([128, 1536], F32, space="PSUM", tag="ps")
                    for i, m, _, col, w_, qlo in pc:
                        nc.tensor.matmul(
                            ps[:, col : col + w_],
                            kt[p0 : p0 + 64, 128 * m : 128 * m + 128],
                            qt[p0 : p0 + 64, q0 + qlo : q0 + qlo + w_],
                            start=True,
                            stop=True,
                            tile_position=(p0, 0),
                        )
                    pt = pt_p.tile([128, 1536], BF16, tag="pt")
                    for a, ln in _merge_ranges(
                        [(col, col + w_) for _, _, _, col, w_, _ in pc]
                    ):
                        nc.scalar.activation(pt[:, a : a + ln], ps[:, a : a + ln], EXP)
                    # diagonal masks: left chunks (i<=3) keep j <= r on the
                    # last 128 cols of their extent, right chunks keep
                    # j >= r on the first 128.
                    moffs = sorted(
                        (col + w_ - 128) if i <= 3 else col
                        for i, _, _, col, w_, _ in pc
                    )
                    mbase = 0 if piece == 0 else 256
                    k = 0
                    while k < len(moffs):
                        nreg = 1
                        while (
                            k + nreg < len(moffs)
                            and moffs[k + nreg] - moffs[k + nreg - 1]
                            == moffs[k + 1] - moffs[k]
                        ):
                            nreg += 1
                        stride = moffs[k + 1] - moffs[k] if nreg > 1 else 128
                        emit_mask(pt, moffs[k], nreg, stride, mbase)
                        k += nreg
                    if has_kmask:
                        for i, m, _, col, w_, qlo in pc:
                            nc.vector.tensor_scalar_mul(
                                pt[:, col : col + w_],
                                pt[:, col : col + w_],
                                kpad_sb[:, m : m + 1],
                            )
                    hpt.append((pt, pc))
                pts.append(hpt)
            blk[s] = pts

        def emit_pv(s):
            q0 = 512 * s
            pts = blk.pop(s)
            # i3 (always full 512-wide) starts the psum group, i4 (also
            # full) stops it; the partial-extent chunks accumulate between.
            pvs = []
            for h in range(3):
                bych = {c[0]: (pc, c) for pc, ch in pts[h] for c in ch}
                order = [3] + [i for i in (0, 1, 2, 5, 6, 7) if i in bych] + [4]
                pv = sm_p.tile([128, 512], F32, space="PSUM", tag="sm")
                for oi, i in enumerate(order):
                    pt, (_, m, _, col, w_, qlo) = bych[i]
                    nc.tensor.matmul(
                        pv[0:65, qlo : qlo + w_],
                        vall[:, m, 65 * h : 65 * h + 65],
                        pt[:, col : col + w_],
                        start=(oi == 0),
                        stop=(oi == len(order) - 1),
                        skip_group_check=True,
                    )
                pvs.append(pv)

            ots = []
            for h in range(3):
                ot = wk_p.tile([65, 512], BF16, name=f"ot{h}")
                nc.vector.tensor_copy(ot[:], pvs[h][0:65, :])
                ots.append(ot)

            # transpose [65, 128] -> [128, 65] per (head, query-quarter);
            # col 64 of each 66-spaced group holds the softmax denominator.
            # Groups 0-6 in psum bank 0, 7-11 in bank 1.
            trp = ps_p.tile([128, 1024], BF16, space="PSUM", tag="ps")
            pos = lambda k: 66 * k if k < 7 else 512 + 66 * (k - 7)
            for h in range(3):
                for g in range(4):
                    k = 4 * h + g
                    nc.tensor.transpose(
                        trp[:, pos(k) : pos(k) + 65],
                        ots[h][0:65, 128 * g : 128 * g + 128],
                        idn_sb[0:65, 0:65],
                    )
            rec = wk_p.tile([128, 16], F32, name="rec")
            for b0, n_, r0 in ((0, 7, 0), (512, 5, 8)):
                dcol = trp[:, b0 : b0 + 66 * n_].rearrange(
                    "p (i x) -> p i x", x=66
                )[:, :, 64:65]
                nc.vector.reciprocal(
                    rec[:, r0 : r0 + n_].rearrange("p (i x) -> p i x", x=1),
                    dcol,
                )
            for g in range(4):
                osb = wk_p.tile([128, 192], F32, name="osb")
                for h in range(3):
                    k = 4 * h + g
                    rc = k if k < 7 else 8 + k - 7
                    nc.vector.tensor_scalar_mul(
                        osb[:, 64 * h : 64 * h + 64],
                        trp[:, pos(k) : pos(k) + 64],
                        rec[:, rc : rc + 1],
                    )
                if has_kmask:
                    nc.vector.tensor_scalar_mul(
                        osb[:], osb[:], qpad_sb[:, 4 * s + g : 4 * s + g + 1]
                    )
                nc.sync.dma_start(
                    out_d[q0 + 128 * g : q0 + 128 * g + 128, 0:192], osb[:]
                )

        # Software pipeline: scores(s) on PE while exp/mask(s-1) drain on
        # Act/Pool, then PV+epilogue(s-1); projection s-tiles interleave.
        emit_proj(0)
        emit_proj(1)
        for i in range(NSB + 1):
            if 1 <= i <= 6:
                emit_proj(i + 1)
            if i < NSB:
                emit_scores(i)
            if i >= 1:
                emit_pv(i - 1)

    nc.compile()
    return nc


_prog_cache = {}


def _get_program(has_bias, has_kmask):
    key = (has_bias, has_kmask)
    if key not in _prog_cache:
        _prog_cache[key] = build_program(has_bias, has_kmask)
    return _prog_cache[key]


def _band_masks():
    """[mL | mL | mR | mR] multiplicative diagonal masks, [128, 512].

    In [key-row r, query-col j] space: mL keeps j <= r (left window edge),
    mR keeps j >= r (right edge); each appears twice so 2-region strided
    mask ops can read consecutive 128-col groups.
    """
    r = np.arange(128)[:, None]
    q = np.arange(128)[None, :]
    mL = (q <= r).astype(np.float32)
    mR = (q >= r).astype(np.float32)
    return np.concatenate([mL, mL, mR, mR], axis=1)


def kernel(hidden_states, attention_mask, Wq, bq, Wk, bk, Wv, bv, _res=[None]):
    hidden_states = np.asarray(hidden_states, np.float32)
    attention_mask = np.asarray(attention_mask, np.float32)
    Wq, Wk, Wv = (np.asarray(w, np.float32) for w in (Wq, Wk, Wv))
    bq, bk, bv = (np.asarray(b_, np.float32) for b_ in (bq, bk, bv))

    scale = 1.0 / np.sqrt(DH)
    has_bias = bool(np.any(bq) or np.any(bk) or np.any(bv))
    has_kmask = bool(np.any(attention_mask < 0))

    hsT = [np.ascontiguousarray(hidden_states[b].T).astype(BF) for b in range(B)]
    masks = _band_masks().astype(BF)
    ident = np.eye(128, dtype=np.float32).astype(BF)
    vones = np.ones((128, 96), BF)
    masked = attention_mask < 0  # [B, S]

    in_maps = []
    for core in range(N_CORES):
        b, h0 = core // 4, (core % 4) * HPC
        sl = slice(h0 * DH, (h0 + HPC) * DH)
        wq = Wq[:, sl] * scale
        wk = Wk[:, sl]
        wqk = np.concatenate(
            [wq[:, 0:128], wk[:, 0:128], wq[:, 128:192], wk[:, 128:192]], axis=1
        )
        m = {
            "hsT": hsT[b],
            "wqk": np.ascontiguousarray(wqk).astype(BF),
            "wv": np.ascontiguousarray(Wv[:, sl]).astype(BF),
            "masks": masks,
            "vones": vones,
            "ident": ident,
        }
        if has_bias:
            bq_s = bq[sl] * scale
            bk_s = bk[sl]
            m["bqk"] = np.concatenate(
                [bq_s[0:128], bk_s[0:128], bq_s[128:192], bk_s[128:192]]
            ).reshape(1, 384).astype(BF)
            m["bv"] = bv[sl].reshape(1, 192).astype(BF)
        if has_kmask:
            keep = (~masked[b]).astype(np.float32).reshape(NKC, 128).T
            m["kpad"] = np.ascontiguousarray(keep)
            m["qpad"] = np.ascontiguousarray(keep)
        in_maps.append(m)

    nc = _get_program(has_bias, has_kmask)
    res = run_bass_kernel_spmd(nc, in_maps, list(range(N_CORES)))
    _res[0] = res

    out = np.empty((B, S, D), np.float32)
    for core in range(N_CORES):
        b, h0 = core // 4, (core % 4) * HPC
        out[b, :, h0 * DH : (h0 + HPC) * DH] = res.results[core]["out"]
    return out


# revision 49
# speedup vs baseline: 1.3435x; 1.0948x over previous
"""Longformer-style windowed self-attention for TRN2, 8-core SPMD.

Sharding: 24 (batch, head) pairs -> 3 heads per core (core c gets batch c//4,
heads (c%4)*3 .. +3). Each core computes QKV projections for its head slice,
windowed attention (block 256, window +-256), and writes its [4096, 192]
output channel slice. Host gathers slices into the full [2, 4096, 768] output.

All matmul inputs are bf16 (psum accumulation fp32). Scores are computed
transposed ([keys, queries]); the softmax window is trimmed: the two outer
key chunks of each 6-chunk window only touch the 128-query half they can
reach, so each head-block does 1280 score columns instead of 1536. Per-head
psum score layout packs chunks as [c1|c0|c5][c4|c2][c3] so the four masked
chunks form one contiguous 768-column region (one multiply on GpSimd) and
exp covers one contiguous 1280-column region (one Activation op).
Renormalization reduces over the partition dim via a ones-column appended to
V; results are PE-transposed back and scaled by the reciprocal row sums.
"""

import sys

for _p in ("/opt/trn_rl_repo", "/opt/pypackages"):
    if _p not in sys.path:
        sys.path.append(_p)

import numpy as np
import ml_dtypes
from contextlib import ExitStack

import concourse.bass as bass
import concourse.bacc as bacc
import concourse.mybir as mybir
import concourse.tile as tile
from concourse.bass_utils import run_bass_kernel_spmd

F32 = mybir.dt.float32
BF16 = mybir.dt.bfloat16
EXP = mybir.ActivationFunctionType.Exp
BF = ml_dtypes.bfloat16

B, S, D = 2, 4096, 768
H, DH = 12, 64
W = 256                 # one-sided window / query block size
NB = S // W             # 16 query blocks
NKC = S // 128          # 32 key chunks of 128
HPC = 3                 # heads per core
N_CORES = 8


NSB = S // 512          # 8 query superblocks of 512

# psum column of chunk i within its piece (piece 0: i<=3, piece 1: i>=4)
_SB_COL = {2: 0, 0: 384, 3: 512, 1: 1024, 4: 0, 5: 512, 7: 896, 6: 1024}


def _sb_chunks(s):
    """Superblock s covers queries [512s, 512s+512); its key window is the
    8 chunks m = 4s-2 .. 4s+5 (chunk position i = m - 4s + 2). Chunk i is
    valid for superblock-relative queries [max(0, 128(i-4)), min(512,
    128(i+1))) — extents 128/256/384/512/512/384/256/128. Left chunks
    (i<=3) are diagonal-masked on the last 128 columns of their extent
    (keep j <= r), right chunks (i>=4) on the first 128 (keep j >= r).

    Returns [(i, m, piece, col, width, qlo)].
    """
    out = []
    for i in range(8):
        m = 4 * s - 2 + i
        if not (0 <= m < NKC):
            continue
        qlo = max(0, 128 * (i - 4))
        qhi = min(512, 128 * (i + 1))
        out.append((i, m, i // 4, _SB_COL[i], qhi - qlo, qlo))
    return out


def _merge_ranges(ivals):
    """Merge sorted [start, end) col intervals into contiguous runs."""
    ivals = sorted(ivals)
    out = [list(ivals[0])]
    for a, b_ in ivals[1:]:
        if a == out[-1][1]:
            out[-1][1] = b_
        else:
            out.append([a, b_])
    return [(a, b_ - a) for a, b_ in out]


def build_program(has_bias, has_kmask):
    nc = bacc.Bacc("TRN2", target_bir_lowering=False, debug=False,
                   num_devices=N_CORES)
    hsT_d = nc.declare_dram_parameter("hsT", [D, S], BF16, isOutput=False)
    wqk_d = nc.declare_dram_parameter("wqk", [D, 384], BF16, isOutput=False)
    wv_d = nc.declare_dram_parameter("wv", [D, 192], BF16, isOutput=False)
    msk_d = nc.declare_dram_parameter("masks", [128, 512], BF16, isOutput=False)
    idn_d = nc.declare_dram_parameter("ident", [128, 128], BF16, isOutput=False)
    if has_bias:
        bqk_d = nc.declare_dram_parameter("bqk", [1, 384], BF16, isOutput=False)
        bv_d = nc.declare_dram_parameter("bv", [1, 192], BF16, isOutput=False)
    if has_kmask:
        kpad_d = nc.declare_dram_parameter("kpad", [128, NKC], F32, isOutput=False)
        qpad_d = nc.declare_dram_parameter("qpad", [128, NKC], F32, isOutput=False)
    out_d = nc.declare_dram_parameter("out", [S, HPC * DH], F32, isOutput=True)

    with tile.TileContext(nc) as tc, ExitStack() as ctx:
        const_p = ctx.enter_context(tc.tile_pool(name="const", bufs=1))
        hst_p = ctx.enter_context(tc.tile_pool(name="hst", bufs=3))
        qkt_p = ctx.enter_context(tc.tile_pool(name="qkt", bufs=1))
        vall_p = ctx.enter_context(tc.tile_pool(name="vall", bufs=1))
        pt_p = ctx.enter_context(tc.tile_pool(name="pt", bufs=12))
        wk_p = ctx.enter_context(tc.tile_pool(name="wk", bufs=16))
        ps_p = ctx.enter_context(tc.tile_pool(name="ps", bufs=2, space="PSUM"))
        sm_p = ctx.enter_context(tc.tile_pool(name="sm", bufs=2, space="PSUM"))

        # ---- constants / weights ----
        wqk_sb = const_p.tile([128, 6, 384], BF16)
        wv_sb = const_p.tile([128, 6, 192], BF16)
        msk_sb = const_p.tile([128, 512], BF16)
        idn_sb = const_p.tile([128, 128], BF16)
        if has_bias:
            bqk_sb = const_p.tile([1, 384], BF16)
            nc.sync.dma_start(bqk_sb[:], bqk_d[:, :])
            bv_sb = const_p.tile([1, 192], BF16)
            nc.sync.dma_start(bv_sb[:], bv_d[:, :])
            ones_sb = const_p.tile([1, 512], BF16)
            nc.vector.memset(ones_sb[:], 1.0)
        if has_kmask:
            kpad_sb = const_p.tile([128, NKC], F32)
            nc.sync.dma_start(kpad_sb[:], kpad_d[:, :])
            qpad_sb = const_p.tile([128, NKC], F32)
            nc.sync.dma_start(qpad_sb[:], qpad_d[:, :])

        # qT/kT for head pair (A,B): A on partitions 0:64, B on 64:128.
        # Head C: qkt_c holds qC on 0:64 / kC on 64:128; qkt_c2[0:64] is a
        # DMA-replicated copy of kC so both score operands sit on 0:64.
        qt_ab = qkt_p.tile([128, S], BF16)
        kt_ab = qkt_p.tile([128, S], BF16)
        qkt_c = qkt_p.tile([128, S], BF16)
        qkt_c2 = qkt_p.tile([64, S], BF16)
        # v in [s, dh] layout: [128, key-chunk, (vA|1|vB|1|vC|1)]
        vall = vall_p.tile([128, NKC, 195], BF16)
        ones_cols = vall[:].rearrange("p m (h x) -> p m h x", h=3)[:, :, :, 64:65]
        nc.vector.memset(ones_cols, 1.0)

        hst_tiles = {}

        def emit_proj_dma(t, split=False):
            hst = hst_p.tile([128, 6, 512], BF16)
            hst_tiles[t] = hst
            s0 = 512 * t
            # hst loads ride the Activation DMA queue so their issue overhead
            # overlaps the SP-queue weight/output transfers. The first tile is
            # split so its first half lands (and projection starts) sooner.
            src = hsT_d[:].rearrange("(c p) s -> p c s", p=128)[:, :, s0 : s0 + 512]
            if split:
                # split on the contraction-chunk dim: the projection's c-loop
                # consumes chunks in order, so matmuls start after the first
                # piece lands.
                nc.sync.dma_start(hst[:, 0:2, :], src[:, 0:2, :])
                nc.sync.dma_start(hst[:, 2:6, :], src[:, 2:6, :])
            else:
                nc.sync.dma_start(hst[:], src)

        def emit_proj_qk(t):
            s0 = 512 * t
            hst = hst_tiles[t]
            # q/k projections: 3 pair-matmuls of M=128 -> [qA|qB], [kA|kB],
            # [qC|kC]
            for j in range(3):
                pp = sm_p.tile([128, 512], F32, space="PSUM", tag="sm")
                for c in range(6):
                    nc.tensor.matmul(
                        pp[:],
                        wqk_sb[:, c, 128 * j : 128 * j + 128],
                        hst[:, c, :],
                        start=(c == 0),
                        stop=(c == 5 and not has_bias),
                    )
                if has_bias:
                    nc.tensor.matmul(
                        pp[:],
                        bqk_sb[0:1, 128 * j : 128 * j + 128],
                        ones_sb[0:1, :],
                        start=False,
                        stop=True,
                    )
                dst = (qt_ab, kt_ab, qkt_c)[j]
                nc.vector.tensor_copy(dst[:, s0 : s0 + 512], pp[:])
            nc.sync.dma_start(qkt_c2[:, s0 : s0 + 512], qkt_c[64:128, s0 : s0 + 512])

        def emit_proj_v(t):
            s0 = 512 * t
            hst = hst_tiles.pop(t)
            # v projection: 4 s-subtiles of 128, packed two per PSUM tile
            for mm0 in (0, 2):
                m = 4 * t + mm0
                pv = sm_p.tile([128, 512], F32, space="PSUM", tag="sm")
                for half, mm in enumerate((mm0, mm0 + 1)):
                    for c in range(6):
                        nc.tensor.matmul(
                            pv[:, 256 * half : 256 * half + 192],
                            hst[:, c, 128 * mm : 128 * mm + 128],
                            wv_sb[:, c, :],
                            start=(c == 0),
                            stop=(c == 5 and not has_bias),
                        )
                    if has_bias:
                        nc.tensor.matmul(
                            pv[:, 256 * half : 256 * half + 192],
                            ones_sb[0:1, 0:128],
                            bv_sb[0:1, :],
                            start=False,
                            stop=True,
                        )
                dst = vall[:, m : m + 2, :].rearrange(
                    "p m (h x) -> p m h x", h=3
                )[:, :, :, 0:64]
                src = pv[:].rearrange("p (m x) -> p m x", m=2)[
                    :, :, 0:192
                ].rearrange("p m (h x) -> p m h x", h=3)
                nc.vector.tensor_copy(dst, src)

        def gpsimd_exp(out, in_):
            """exp on the GpSimd engine (InstActivation emitted manually —
            bass only exposes activation on the scalar engine)."""
            g = nc.gpsimd
            bias = g.bass.const_aps.scalar_like(0.0, in_)
            return g.add_instruction(
                mybir.InstActivation(
                    name=g.bass.get_next_instruction_name(),
                    func=EXP,
                    ins=[
                        g.lower_ap(in_),
                        g.lower_ap(bias),
                        mybir.ImmediateValue(dtype=mybir.dt.float32, value=1.0),
                        mybir.ImmediateValue(dtype=mybir.dt.float32, value=0.0),
                    ],
                    outs=[g.lower_ap(out)],
                )
            )

        def emit_mask(pt, in_off, nreg, stride, msk_off):
            """pt[:, in_off + k*stride : +128] *= msk[:, msk_off + k*128]
            for k in range(nreg), as one strided TensorTensor."""
            if nreg == 1:
                in_ap = pt[:, in_off : in_off + 128]
                mk_ap = msk_sb[:, msk_off : msk_off + 128]
            else:
                ln = stride * (nreg - 1) + 128
                in_ap = pt[:, in_off : in_off + ln].rearrange(
                    "p (a x) -> p a x", x=128
                )[:, :: stride // 128, :]
                mk_ap = msk_sb[:, msk_off : msk_off + 128 * nreg].rearrange(
                    "p (a x) -> p a x", x=128
                )
            nc.gpsimd.tensor_mul(in_ap, in_ap, mk_ap)

        # per-superblock state flowing scores -> PV -> epilogue
        blk = {}

        def emit_scores_head(s, h):
            q0 = 512 * s
            chunks = _sb_chunks(s)
            if h == 0:
                kt, qt, p0 = kt_ab, qt_ab, 0
            elif h == 1:
                kt, qt, p0 = kt_ab, qt_ab, 64
            else:
                kt, qt, p0 = qkt_c2, qkt_c, 0
            hpt = []
            blk.setdefault(s, {"pts": [], "ots": []})["pts"].append(hpt)
            if True:
                for piece in range(2):
                    pc = [c for c in chunks if c[2] == piece]
                    ps = ps_p.tile([128, 1536], F32, space="PSUM", tag="ps")
                    for i, m, _, col, w_, qlo in pc:
                        nc.tensor.matmul(
                            ps[:, col : col + w_],
                            kt[p0 : p0 + 64, 128 * m : 128 * m + 128],
                            qt[p0 : p0 + 64, q0 + qlo : q0 + qlo + w_],
                            start=True,
                            stop=True,
                            tile_position=(p0, 0),
                        )
                    pt = pt_p.tile([128, 1536], BF16, tag="pt")
                    for a, ln in _merge_ranges(
                        [(col, col + w_) for _, _, _, col, w_, _ in pc]
                    ):
                        nc.scalar.activation(pt[:, a : a + ln], ps[:, a : a + ln], EXP)
                    # diagonal masks: left chunks (i<=3) keep j <= r on the
                    # last 128 cols of their extent, right chunks keep
                    # j >= r on the first 128.
                    moffs = sorted(
                        (col + w_ - 128) if i <= 3 else col
                        for i, _, _, col, w_, _ in pc
                    )
                    mbase = 0 if piece == 0 else 256
                    k = 0
                    while k < len(moffs):
                        nreg = 1
                        while (
                            k + nreg < len(moffs)
                            and moffs[k + nreg] - moffs[k + nreg - 1]
                            == moffs[k + 1] - moffs[k]
                        ):
                            nreg += 1
                        stride = moffs[k + 1] - moffs[k] if nreg > 1 else 128
                        emit_mask(pt, moffs[k], nreg, stride, mbase)
                        k += nreg
                    if has_kmask:
                        for i, m, _, col, w_, qlo in pc:
                            nc.vector.tensor_scalar_mul(
                                pt[:, col : col + w_],
                                pt[:, col : col + w_],
                                kpad_sb[:, m : m + 1],
                            )
                    hpt.append((pt, pc))

        def emit_pv_head(s, h):
            # i3 (always full 512-wide) starts the psum group, i4 (also
            # full) stops it; partial-extent chunks accumulate between.
            st = blk[s]
            bych = {c[0]: (pc, c) for pc, ch in st["pts"][h] for c in ch}
            order = [3] + [i for i in (0, 1, 2, 5, 6, 7) if i in bych] + [4]
            pv = sm_p.tile([128, 512], F32, space="PSUM", tag="sm")
            for oi, i in enumerate(order):
                pt, (_, m, _, col, w_, qlo) = bych[i]
                nc.tensor.matmul(
                    pv[0:65, qlo : qlo + w_],
                    vall[:, m, 65 * h : 65 * h + 65],
                    pt[:, col : col + w_],
                    start=(oi == 0),
                    stop=(oi == len(order) - 1),
                    skip_group_check=True,
                )
            ot = wk_p.tile([65, 512], BF16, name=f"ot{h}")
            nc.vector.tensor_copy(ot[:], pv[0:65, :])
            st["ots"].append(ot)

        def emit_epi_head(s, h):
            # Epilogue: transpose head h's [65, 512] into trp bank h
            # (66-spaced query-quarters; col 64 of each group is the softmax
            # denominator), then scale by the reciprocal row sums.
            st = blk[s]
            if "trp" not in st:
                st["trp"] = ps_p.tile(
                    [128, 1536], BF16, space="PSUM", tag="ps", name="trp"
                )
                st["rec"] = wk_p.tile([128, 16], F32, name="rec")
                st["osbs"] = [wk_p.tile([128, 192], F32, name="osb") for _ in range(4)]
            trp, rec, osbs = st["trp"], st["rec"], st["osbs"]
            for g in range(4):
                nc.tensor.transpose(
                    trp[:, 512 * h + 66 * g : 512 * h + 66 * g + 65],
                    st["ots"][h][0:65, 128 * g : 128 * g + 128],
                    idn_sb[0:65, 0:65],
                )
            dcol = trp[:, 512 * h : 512 * h + 264].rearrange(
                "p (i x) -> p i x", x=66
            )[:, :, 64:65]
            nc.vector.reciprocal(
                rec[:, 4 * h : 4 * h + 4].rearrange("p (i x) -> p i x", x=1),
                dcol,
            )
            for g in range(4):
                args = (
                    osbs[g][:, 64 * h : 64 * h + 64],
                    trp[:, 512 * h + 66 * g : 512 * h + 66 * g + 64],
                    rec[:, 4 * h + g : 4 * h + g + 1],
                )
                # Act engine helps drain the final epilogue, where no more
                # exps compete for it.
                if s == NSB - 1 and g % 2:
                    nc.scalar.mul(*args)
                else:
                    nc.vector.tensor_scalar_mul(*args)

        def emit_epi_out(s):
            q0 = 512 * s
            st = blk.pop(s)
            for g in range(4):
                if has_kmask:
                    nc.vector.tensor_scalar_mul(
                        st["osbs"][g][:], st["osbs"][g][:],
                        qpad_sb[:, 4 * s + g : 4 * s + g + 1],
                    )
                nc.sync.dma_start(
                    out_d[q0 + 128 * g : q0 + 128 * g + 128, 0:192],
                    st["osbs"][g][:],
                )

        # Software pipeline: scores(s) on PE while exp/mask(s-1) drain on
        # Act/Pool, then PV+epilogue(s-1); projection work interleaves.
        # scores(i) needs qk through tile i+1; pv(i-1) needs v through tile
        # i. DMA order front-loads what the first matmuls need: wqk, hst(0),
        # then the rest of the constants.
        wqk_src = wqk_d[:].rearrange("(c p) n -> p c n", p=128)
        nc.sync.dma_start(wqk_sb[:, :, 0:128], wqk_src[:, :, 0:128])
        nc.sync.dma_start(wqk_sb[:, :, 128:384], wqk_src[:, :, 128:384])
        emit_proj_dma(0, split=True)
        nc.sync.dma_start(wv_sb[:], wv_d[:].rearrange("(c p) n -> p c n", p=128))
        emit_proj_qk(0)
        nc.sync.dma_start(msk_sb[:], msk_d[:, :])
        nc.sync.dma_start(idn_sb[:], idn_d[:, :])
        emit_proj_dma(1)
        emit_proj_qk(1)
        emit_proj_v(0)
        emit_proj_dma(2)
        # PV heads of superblock i-1 interleave between score heads of i so
        # PE has work while Act drains each head's exp (psum slot lockstep).
        # trp(i-1) must allocate after scores(i)'s psum tiles (epi after all
        # scores) or a later score tile would evict it before its readers.
        for i in range(NSB):
            if i + 3 <= 7:
                emit_proj_dma(i + 3)
            if i + 2 <= 7:
                emit_proj_qk(i + 2)
            for h in range(3):
                emit_scores_head(i, h)
                if i >= 1:
                    emit_pv_head(i - 1, h)
            if i >= 1:
                for h in range(3):
                    emit_epi_head(i - 1, h)
                emit_epi_out(i - 1)
            if 1 <= i + 1 <= 7:
                emit_proj_v(i + 1)
        # final superblock: no scores left, so the per-head epilogue chains
        # interleave directly with the remaining PV heads.
        s = NSB - 1
        emit_pv_head(s, 0)
        emit_pv_head(s, 1)
        emit_epi_head(s, 0)
        emit_pv_head(s, 2)
        emit_epi_head(s, 1)
        emit_epi_head(s, 2)
        emit_epi_out(s)

    nc.compile()
    return nc


_prog_cache = {}


def _get_program(has_bias, has_kmask):
    key = (has_bias, has_kmask)
    if key not in _prog_cache:
        _prog_cache[key] = build_program(has_bias, has_kmask)
    return _prog_cache[key]


def _band_masks():
    """[mL | mL | mR | mR] multiplicative diagonal masks, [128, 512].

    In [key-row r, query-col j] space: mL keeps j <= r (left window edge),
    mR keeps j >= r (right edge); each appears twice so 2-region strided
    mask ops can read consecutive 128-col groups.
    """
    r = np.arange(128)[:, None]
    q = np.arange(128)[None, :]
    mL = (q <= r).astype(np.float32)
    mR = (q >= r).astype(np.float32)
    return np.concatenate([mL, mL, mR, mR], axis=1)


def kernel(hidden_states, attention_mask, Wq, bq, Wk, bk, Wv, bv, _res=[None]):
    hidden_states = np.asarray(hidden_states, np.float32)
    attention_mask = np.asarray(attention_mask, np.float32)
    Wq, Wk, Wv = (np.asarray(w, np.float32) for w in (Wq, Wk, Wv))
    bq, bk, bv = (np.asarray(b_, np.float32) for b_ in (bq, bk, bv))

    scale = 1.0 / np.sqrt(DH)
    has_bias = bool(np.any(bq) or np.any(bk) or np.any(bv))
    has_kmask = bool(np.any(attention_mask < 0))

    hsT = [np.ascontiguousarray(hidden_states[b].T).astype(BF) for b in range(B)]
    masks = _band_masks().astype(BF)
    ident = np.eye(128, dtype=np.float32).astype(BF)
    masked = attention_mask < 0  # [B, S]

    in_maps = []
    for core in range(N_CORES):
        b, h0 = core // 4, (core % 4) * HPC
        sl = slice(h0 * DH, (h0 + HPC) * DH)
        wq = Wq[:, sl] * scale
        wk = Wk[:, sl]
        wqk = np.concatenate(
            [wq[:, 0:128], wk[:, 0:128], wq[:, 128:192], wk[:, 128:192]], axis=1
        )
        m = {
            "hsT": hsT[b],
            "wqk": np.ascontiguousarray(wqk).astype(BF),
            "wv": np.ascontiguousarray(Wv[:, sl]).astype(BF),
            "masks": masks,
            "ident": ident,
        }
        if has_bias:
            bq_s = bq[sl] * scale
            bk_s = bk[sl]
            m["bqk"] = np.concatenate(
                [bq_s[0:128], bk_s[0:128], bq_s[128:192], bk_s[128:192]]
            ).reshape(1, 384).astype(BF)
            m["bv"] = bv[sl].reshape(1, 192).astype(BF)
        if has_kmask:
            keep = (~masked[b]).astype(np.float32).reshape(NKC, 128).T
            m["kpad"] = np.ascontiguousarray(keep)
            m["qpad"] = np.ascontiguousarray(keep)
        in_maps.append(m)

    nc = _get_program(has_bias, has_kmask)
    res = run_bass_kernel_spmd(nc, in_maps, list(range(N_CORES)))
    _res[0] = res

    out = np.empty((B, S, D), np.float32)
    for core in range(N_CORES):
        b, h0 = core // 4, (core % 4) * HPC
        out[b, :, h0 * DH : (h0 + HPC) * DH] = res.results[core]["out"]
    return out


# revision 74
# speedup vs baseline: 1.3739x; 1.0226x over previous
"""Longformer-style windowed self-attention for TRN2, 8-core SPMD.

Sharding: 24 (batch, head) pairs -> 3 heads per core (core c gets batch c//4,
heads (c%4)*3 .. +3). Each core computes QKV projections for its head slice,
windowed attention (block 256, window +-256), and writes its [4096, 192]
output channel slice. Host gathers slices into the full [2, 4096, 768] output.

All matmul inputs are bf16 (psum accumulation fp32). Scores are computed
transposed ([keys, queries]); the softmax window is trimmed: the two outer
key chunks of each 6-chunk window only touch the 128-query half they can
reach, so each head-block does 1280 score columns instead of 1536. Per-head
psum score layout packs chunks as [c1|c0|c5][c4|c2][c3] so the four masked
chunks form one contiguous 768-column region (one multiply on GpSimd) and
exp covers one contiguous 1280-column region (one Activation op).
Renormalization reduces over the partition dim via a ones-column appended to
V; results are PE-transposed back and scaled by the reciprocal row sums.
"""

import sys

for _p in ("/opt/trn_rl_repo", "/opt/pypackages"):
    if _p not in sys.path:
        sys.path.append(_p)

import numpy as np
import ml_dtypes
from contextlib import ExitStack

import concourse.bass as bass
import concourse.bacc as bacc
import concourse.mybir as mybir
import concourse.tile as tile
from concourse.bass_utils import run_bass_kernel_spmd

F32 = mybir.dt.float32
R32 = mybir.dt.float32r
BF16 = mybir.dt.bfloat16
EXP = mybir.ActivationFunctionType.Exp
BF = ml_dtypes.bfloat16

B, S, D = 2, 4096, 768
H, DH = 12, 64
W = 256                 # one-sided window / query block size
NB = S // W             # 16 query blocks
NKC = S // 128          # 32 key chunks of 128
HPC = 3                 # heads per core
N_CORES = 8


NSB = S // 512          # 8 query superblocks of 512

# psum column of chunk i within its piece (piece 0: i<=3, piece 1: i>=4)
_SB_COL = {2: 0, 0: 384, 3: 512, 1: 1024, 4: 0, 5: 512, 7: 896, 6: 1024}


def _sb_chunks(s):
    """Superblock s covers queries [512s, 512s+512); its key window is the
    8 chunks m = 4s-2 .. 4s+5 (chunk position i = m - 4s + 2). Chunk i is
    valid for superblock-relative queries [max(0, 128(i-4)), min(512,
    128(i+1))) — extents 128/256/384/512/512/384/256/128. Left chunks
    (i<=3) are diagonal-masked on the last 128 columns of their extent
    (keep j <= r), right chunks (i>=4) on the first 128 (keep j >= r).

    Returns [(i, m, piece, col, width, qlo)].
    """
    out = []
    for i in range(8):
        m = 4 * s - 2 + i
        if not (0 <= m < NKC):
            continue
        qlo = max(0, 128 * (i - 4))
        qhi = min(512, 128 * (i + 1))
        out.append((i, m, i // 4, _SB_COL[i], qhi - qlo, qlo))
    return out


def _merge_ranges(ivals):
    """Merge sorted [start, end) col intervals into contiguous runs."""
    ivals = sorted(ivals)
    out = [list(ivals[0])]
    for a, b_ in ivals[1:]:
        if a == out[-1][1]:
            out[-1][1] = b_
        else:
            out.append([a, b_])
    return [(a, b_ - a) for a, b_ in out]


def build_program(has_bias, has_kmask):
    nc = bacc.Bacc("TRN2", target_bir_lowering=False, debug=False,
                   num_devices=N_CORES)
    hsT_d = nc.declare_dram_parameter("hsT", [D, S], BF16, isOutput=False)
    wqk_d = nc.declare_dram_parameter("wqk", [D, 384], BF16, isOutput=False)
    wv_d = nc.declare_dram_parameter("wv", [D, 192], BF16, isOutput=False)
    msk_d = nc.declare_dram_parameter("masks", [128, 512], BF16, isOutput=False)
    idn_d = nc.declare_dram_parameter("ident", [128, 128], BF16, isOutput=False)
    if has_bias:
        bqk_d = nc.declare_dram_parameter("bqk", [1, 384], BF16, isOutput=False)
        bv_d = nc.declare_dram_parameter("bv", [1, 192], BF16, isOutput=False)
    if has_kmask:
        kpad_d = nc.declare_dram_parameter("kpad", [128, NKC], F32, isOutput=False)
        qpad_d = nc.declare_dram_parameter("qpad", [128, NKC], F32, isOutput=False)
    out_d = nc.declare_dram_parameter("out", [S, HPC * DH], F32, isOutput=True)

    with tile.TileContext(nc) as tc, ExitStack() as ctx:
        const_p = ctx.enter_context(tc.tile_pool(name="const", bufs=1))
        hst_p = ctx.enter_context(tc.tile_pool(name="hst", bufs=3))
        qkt_p = ctx.enter_context(tc.tile_pool(name="qkt", bufs=1))
        vall_p = ctx.enter_context(tc.tile_pool(name="vall", bufs=1))
        pt_p = ctx.enter_context(tc.tile_pool(name="pt", bufs=24))
        wk_p = ctx.enter_context(tc.tile_pool(name="wk", bufs=16))
        ps_p = ctx.enter_context(tc.tile_pool(name="ps", bufs=2, space="PSUM"))
        sm_p = ctx.enter_context(tc.tile_pool(name="sm", bufs=2, space="PSUM"))

        # ---- constants / weights ----
        wqk_sb = const_p.tile([128, 6, 384], BF16)
        wv_sb = const_p.tile([128, 6, 192], BF16)
        msk_sb = const_p.tile([128, 512], BF16)
        idn_sb = const_p.tile([128, 128], BF16)
        nc.sync.dma_start(idn_sb[:], idn_d[:, :])
        if has_bias:
            bqk_sb = const_p.tile([1, 384], BF16)
            nc.sync.dma_start(bqk_sb[:], bqk_d[:, :])
            bv_sb = const_p.tile([1, 192], BF16)
            nc.sync.dma_start(bv_sb[:], bv_d[:, :])
            ones_sb = const_p.tile([1, 512], BF16)
            nc.vector.memset(ones_sb[:], 1.0)
        if has_kmask:
            kpad_sb = const_p.tile([128, NKC], F32)
            nc.sync.dma_start(kpad_sb[:], kpad_d[:, :])
            qpad_sb = const_p.tile([128, NKC], F32)
            nc.sync.dma_start(qpad_sb[:], qpad_d[:, :])

        # qT/kT for head pair (A,B): A on partitions 0:64, B on 64:128.
        # Head C: qkt_c holds qC on 0:64 / kC on 64:128; qkt_c2[0:64] is a
        # DMA-replicated copy of kC so both score operands sit on 0:64.
        qt_ab = qkt_p.tile([128, S], BF16)
        kt_ab = qkt_p.tile([128, S], BF16)
        qkt_c = qkt_p.tile([128, S], BF16)
        qkt_c2 = qkt_p.tile([64, S], BF16)
        # v in [s, dh] layout: [128, key-chunk, (vA|1|vB|1|vC|1)]
        vall = vall_p.tile([128, NKC, 195], BF16)
        ones_cols = vall[:].rearrange("p m (h x) -> p m h x", h=3)[:, :, :, 64:65]
        nc.vector.memset(ones_cols, 1.0)

        hst_tiles = {}

        def emit_proj_dma(t, split=False):
            hst = hst_p.tile([128, 6, 512], BF16)
            hst_tiles[t] = hst
            s0 = 512 * t
            # hst loads ride the Activation DMA queue so their issue overhead
            # overlaps the SP-queue weight/output transfers. The first tile is
            # split so its first half lands (and projection starts) sooner.
            src = hsT_d[:].rearrange("(c p) s -> p c s", p=128)[:, :, s0 : s0 + 512]
            if split:
                # split on the contraction-chunk dim: the projection's c-loop
                # consumes chunks in order, so matmuls start after the first
                # piece lands.
                nc.sync.dma_start(hst[:, 0:2, :], src[:, 0:2, :])
                nc.sync.dma_start(hst[:, 2:6, :], src[:, 2:6, :])
            else:
                nc.sync.dma_start(hst[:], src)

        def emit_proj_qk(t):
            s0 = 512 * t
            hst = hst_tiles[t]
            # q/k projections: 3 pair-matmuls of M=128 -> [qA|qB], [kA|kB],
            # [qC|kC]
            for j in range(3):
                pp = sm_p.tile([128, 512], F32, space="PSUM", tag="sm")
                for c in range(6):
                    nc.tensor.matmul(
                        pp[:],
                        wqk_sb[:, c, 128 * j : 128 * j + 128],
                        hst[:, c, :],
                        start=(c == 0),
                        stop=(c == 5 and not has_bias),
                    )
                if has_bias:
                    nc.tensor.matmul(
                        pp[:],
                        bqk_sb[0:1, 128 * j : 128 * j + 128],
                        ones_sb[0:1, :],
                        start=False,
                        stop=True,
                    )
                dst = (qt_ab, kt_ab, qkt_c)[j]
                nc.vector.tensor_copy(dst[:, s0 : s0 + 512], pp[:])
            nc.sync.dma_start(qkt_c2[:, s0 : s0 + 512], qkt_c[64:128, s0 : s0 + 512])

        def emit_proj_v(t):
            s0 = 512 * t
            hst = hst_tiles.pop(t)
            # v projection: 4 s-subtiles of 128, packed two per PSUM tile
            for mm0 in (0, 2):
                m = 4 * t + mm0
                pv = sm_p.tile([128, 512], F32, space="PSUM", tag="sm")
                for half, mm in enumerate((mm0, mm0 + 1)):
                    for c in range(6):
                        nc.tensor.matmul(
                            pv[:, 256 * half : 256 * half + 192],
                            hst[:, c, 128 * mm : 128 * mm + 128],
                            wv_sb[:, c, :],
                            start=(c == 0),
                            stop=(c == 5 and not has_bias),
                        )
                    if has_bias:
                        nc.tensor.matmul(
                            pv[:, 256 * half : 256 * half + 192],
                            ones_sb[0:1, 0:128],
                            bv_sb[0:1, :],
                            start=False,
                            stop=True,
                        )
                dst = vall[:, m : m + 2, :].rearrange(
                    "p m (h x) -> p m h x", h=3
                )[:, :, :, 0:64]
                src = pv[:].rearrange("p (m x) -> p m x", m=2)[
                    :, :, 0:192
                ].rearrange("p m (h x) -> p m h x", h=3)
                nc.vector.tensor_copy(dst, src)

        def gpsimd_exp(out, in_):
            """exp on the GpSimd engine (InstActivation emitted manually —
            bass only exposes activation on the scalar engine)."""
            g = nc.gpsimd
            bias = g.bass.const_aps.scalar_like(0.0, in_)
            return g.add_instruction(
                mybir.InstActivation(
                    name=g.bass.get_next_instruction_name(),
                    func=EXP,
                    ins=[
                        g.lower_ap(in_),
                        g.lower_ap(bias),
                        mybir.ImmediateValue(dtype=mybir.dt.float32, value=1.0),
                        mybir.ImmediateValue(dtype=mybir.dt.float32, value=0.0),
                    ],
                    outs=[g.lower_ap(out)],
                )
            )

        def emit_mask(pt, in_off, nreg, stride, msk_off):
            """pt[:, in_off + k*stride : +128] *= msk[:, msk_off + k*128]
            for k in range(nreg), as one strided TensorTensor."""
            if nreg == 1:
                in_ap = pt[:, in_off : in_off + 128]
                mk_ap = msk_sb[:, msk_off : msk_off + 128]
            else:
                ln = stride * (nreg - 1) + 128
                in_ap = pt[:, in_off : in_off + ln].rearrange(
                    "p (a x) -> p a x", x=128
                )[:, :: stride // 128, :]
                mk_ap = msk_sb[:, msk_off : msk_off + 128 * nreg].rearrange(
                    "p (a x) -> p a x", x=128
                )
            nc.gpsimd.tensor_mul(in_ap, in_ap, mk_ap)

        # per-superblock state flowing scores -> PV -> epilogue
        blk = {}

        def emit_scores_head(s, h):
            q0 = 512 * s
            chunks = _sb_chunks(s)
            if h == 0:
                kt, qt, p0 = kt_ab, qt_ab, 0
            elif h == 1:
                kt, qt, p0 = kt_ab, qt_ab, 64
            else:
                kt, qt, p0 = qkt_c2, qkt_c, 0
            hpt = []
            blk.setdefault(s, {"pts": [], "ots": []})["pts"].append(hpt)
            if True:
                for piece in range(2):
                    pc = [c for c in chunks if c[2] == piece]
                    ps = ps_p.tile([128, 1536], F32, space="PSUM", tag="ps")
                    for i, m, _, col, w_, qlo in pc:
                        nc.tensor.matmul(
                            ps[:, col : col + w_],
                            kt[p0 : p0 + 64, 128 * m : 128 * m + 128],
                            qt[p0 : p0 + 64, q0 + qlo : q0 + qlo + w_],
                            start=True,
                            stop=True,
                            tile_position=(p0, 0),
                        )
                    pt = pt_p.tile([128, 1536], BF16, tag="pt")
                    for a, ln in _merge_ranges(
                        [(col, col + w_) for _, _, _, col, w_, _ in pc]
                    ):
                        nc.scalar.activation(pt[:, a : a + ln], ps[:, a : a + ln], EXP)
                    # diagonal masks: left chunks (i<=3) keep j <= r on the
                    # last 128 cols of their extent, right chunks keep
                    # j >= r on the first 128.
                    moffs = sorted(
                        (col + w_ - 128) if i <= 3 else col
                        for i, _, _, col, w_, _ in pc
                    )
                    mbase = 0 if piece == 0 else 256
                    k = 0
                    while k < len(moffs):
                        nreg = 1
                        while (
                            k + nreg < len(moffs)
                            and moffs[k + nreg] - moffs[k + nreg - 1]
                            == moffs[k + 1] - moffs[k]
                        ):
                            nreg += 1
                        stride = moffs[k + 1] - moffs[k] if nreg > 1 else 128
                        emit_mask(pt, moffs[k], nreg, stride, mbase)
                        k += nreg
                    if has_kmask:
                        for i, m, _, col, w_, qlo in pc:
                            nc.vector.tensor_scalar_mul(
                                pt[:, col : col + w_],
                                pt[:, col : col + w_],
                                kpad_sb[:, m : m + 1],
                            )
                    hpt.append((pt, pc))

        def emit_pv_head(s, h):
            # i3 (always full 512-wide) starts the psum group, i4 (also
            # full) stops it; partial-extent chunks accumulate between.
            st = blk[s]
            bych = {c[0]: (pc, c) for pc, ch in st["pts"][h] for c in ch}
            order = [3] + [i for i in (0, 1, 2, 5, 6, 7) if i in bych] + [4]
            pv = sm_p.tile([128, 512], F32, space="PSUM", tag="sm")
            for oi, i in enumerate(order):
                pt, (_, m, _, col, w_, qlo) = bych[i]
                nc.tensor.matmul(
                    pv[0:65, qlo : qlo + w_],
                    vall[:, m, 65 * h : 65 * h + 65],
                    pt[:, col : col + w_],
                    start=(oi == 0),
                    stop=(oi == len(order) - 1),
                    skip_group_check=True,
                )
            ot = wk_p.tile([65, 512], BF16, name=f"ot{h}")
            # tail-phase epilogues split per head across DVE/Act/GpSimd
            if s >= NSB - 3 and h >= 1:
                nc.scalar.copy(ot[:], pv[0:65, :])
            else:
                nc.vector.tensor_copy(ot[:], pv[0:65, :])
            st["ots"].append(ot)

        def emit_epi_head(s, h):
            # Epilogue: transpose head h's [65, 512] into trp bank h
            # (66-spaced query-quarters; col 64 of each group is the softmax
            # denominator), then scale by the reciprocal row sums.
            st = blk[s]
            if "trp" not in st:
                st["trp"] = ps_p.tile(
                    [128, 1536], BF16, space="PSUM", tag="ps", name="trp"
                )
                st["rec"] = wk_p.tile([128, 16], F32, name="rec")
                st["osbs"] = [
                    wk_p.tile([128, 192], F32, name="osb") for _ in range(4)
                ]
            trp, rec, osbs = st["trp"], st["rec"], st["osbs"]
            for g in range(4):
                nc.tensor.transpose(
                    trp[:, 512 * h + 66 * g : 512 * h + 66 * g + 65],
                    st["ots"][h][0:65, 128 * g : 128 * g + 128],
                    idn_sb[0:65, 0:65],
                )
            dcol = trp[:, 512 * h : 512 * h + 264].rearrange(
                "p (i x) -> p i x", x=66
            )[:, :, 64:65]
            nc.vector.reciprocal(
                rec[:, 4 * h : 4 * h + 4].rearrange("p (i x) -> p i x", x=1),
                dcol,
            )
            for g in range(4):
                args = (
                    osbs[g][:, 64 * h : 64 * h + 64],
                    trp[:, 512 * h + 66 * g : 512 * h + 66 * g + 64],
                    rec[:, 4 * h + g : 4 * h + g + 1],
                )
                # Act and GpSimd drain the tail-phase epilogues, where
                # no more exps or masks compete for them.
                if s >= NSB - 3 and h >= 1:
                    nc.scalar.mul(*args)
                else:
                    nc.vector.tensor_scalar_mul(*args)

        def emit_epi_out(s):
            q0 = 512 * s
            st = blk.pop(s)
            for g in range(4):
                if has_kmask:
                    nc.vector.tensor_scalar_mul(
                        st["osbs"][g][:], st["osbs"][g][:],
                        qpad_sb[:, 4 * s + g : 4 * s + g + 1],
                    )
                nc.sync.dma_start(
                    out_d[q0 + 128 * g : q0 + 128 * g + 128, 0:192],
                    st["osbs"][g][:],
                )

        # Software pipeline: scores(s) on PE while exp/mask(s-1) drain on
        # Act/Pool, then PV+epilogue(s-1); projection work interleaves.
        # scores(i) needs qk through tile i+1; pv(i-1) needs v through tile
        # i. DMA order front-loads what the first matmuls need: wqk, hst(0),
        # then the rest of the constants.
        wqk_src = wqk_d[:].rearrange("(c p) n -> p c n", p=128)
        nc.sync.dma_start(wqk_sb[:, 0:2, :], wqk_src[:, 0:2, :])
        emit_proj_dma(0, split=True)
        nc.sync.dma_start(wqk_sb[:, 2:6, :], wqk_src[:, 2:6, :])
        nc.sync.dma_start(wv_sb[:], wv_d[:].rearrange("(c p) n -> p c n", p=128))
        emit_proj_qk(0)
        nc.sync.dma_start(msk_sb[:], msk_d[:, :])
        emit_proj_dma(1)
        emit_proj_qk(1)
        emit_proj_v(0)
        emit_proj_dma(2)
        # PV+epilogue trail scores by PVLAG+1 superblocks: the deferred PV
        # work fills PE during the post-projection iterations where scores
        # would otherwise lockstep with the Act engine's exp drain (psum
        # score slots only free once exp'd). trp(p) must allocate after
        # scores(i)'s psum tiles or a later score tile would evict it
        # before its readers.
        PVLAG = 2
        for i in range(NSB + PVLAG + 1):
            p = i - PVLAG - 1
            if i + 3 <= 7:
                emit_proj_dma(i + 3)
            if i + 2 <= 7:
                emit_proj_qk(i + 2)
            if i == NSB + PVLAG:
                # last iteration: interleave the epilogue chains between the
                # remaining PV heads so the end chain starts sooner
                emit_pv_head(p, 0)
                emit_pv_head(p, 1)
                emit_epi_head(p, 0)
                emit_pv_head(p, 2)
                emit_epi_head(p, 1)
                emit_epi_head(p, 2)
                emit_epi_out(p)
            else:
                for h in range(3):
                    if i < NSB:
                        emit_scores_head(i, h)
                    if p >= 0:
                        emit_pv_head(p, h)
                if p >= 0:
                    for h in range(3):
                        emit_epi_head(p, h)
                    emit_epi_out(p)
            if 1 <= i + 1 <= 7:
                emit_proj_v(i + 1)

    nc.compile()
    return nc


_prog_cache = {}


def _get_program(has_bias, has_kmask):
    key = (has_bias, has_kmask)
    if key not in _prog_cache:
        _prog_cache[key] = build_program(has_bias, has_kmask)
    return _prog_cache[key]


def _band_masks():
    """[mL | mL | mR | mR] multiplicative diagonal masks, [128, 512].

    In [key-row r, query-col j] space: mL keeps j <= r (left window edge),
    mR keeps j >= r (right edge); each appears twice so 2-region strided
    mask ops can read consecutive 128-col groups.
    """
    r = np.arange(128)[:, None]
    q = np.arange(128)[None, :]
    mL = (q <= r).astype(np.float32)
    mR = (q >= r).astype(np.float32)
    return np.concatenate([mL, mL, mR, mR], axis=1)


def kernel(hidden_states, attention_mask, Wq, bq, Wk, bk, Wv, bv, _res=[None]):
    hidden_states = np.asarray(hidden_states, np.float32)
    attention_mask = np.asarray(attention_mask, np.float32)
    Wq, Wk, Wv = (np.asarray(w, np.float32) for w in (Wq, Wk, Wv))
    bq, bk, bv = (np.asarray(b_, np.float32) for b_ in (bq, bk, bv))

    scale = 1.0 / np.sqrt(DH)
    has_bias = bool(np.any(bq) or np.any(bk) or np.any(bv))
    has_kmask = bool(np.any(attention_mask < 0))

    hsT = [np.ascontiguousarray(hidden_states[b].T).astype(BF) for b in range(B)]
    masks = _band_masks().astype(BF)
    ident = np.eye(128, dtype=np.float32).astype(BF)
    ident = np.eye(128, dtype=np.float32).astype(BF)
    masked = attention_mask < 0  # [B, S]

    in_maps = []
    for core in range(N_CORES):
        b, h0 = core // 4, (core % 4) * HPC
        sl = slice(h0 * DH, (h0 + HPC) * DH)
        wq = Wq[:, sl] * scale
        wk = Wk[:, sl]
        wqk = np.concatenate(
            [wq[:, 0:128], wk[:, 0:128], wq[:, 128:192], wk[:, 128:192]], axis=1
        )
        m = {
            "hsT": hsT[b],
            "wqk": np.ascontiguousarray(wqk).astype(BF),
            "wv": np.ascontiguousarray(Wv[:, sl]).astype(BF),
            "masks": masks,
            "ident": ident,
        }
        if has_bias:
            bq_s = bq[sl] * scale
            bk_s = bk[sl]
            m["bqk"] = np.concatenate(
                [bq_s[0:128], bk_s[0:128], bq_s[128:192], bk_s[128:192]]
            ).reshape(1, 384).astype(BF)
            m["bv"] = bv[sl].reshape(1, 192).astype(BF)
        if has_kmask:
            keep = (~masked[b]).astype(np.float32).reshape(NKC, 128).T
            m["kpad"] = np.ascontiguousarray(keep)
            m["qpad"] = np.ascontiguousarray(keep)
        in_maps.append(m)

    nc = _get_program(has_bias, has_kmask)
    res = run_bass_kernel_spmd(nc, in_maps, list(range(N_CORES)))
    _res[0] = res

    out = np.empty((B, S, D), np.float32)
    for core in range(N_CORES):
        b, h0 = core // 4, (core % 4) * HPC
        out[b, :, h0 * DH : (h0 + HPC) * DH] = res.results[core]["out"]
    return out
